# revision 1
# baseline (speedup 1.0000x reference)
"""MLA (multi-head latent attention) Bass kernel for Trainium2, 8 NeuronCores.

Sharding: core i handles batch b = i // 2 and head-group g = i % 2
(8 of the 16 heads).  Each core computes a partial output
(its heads' contribution through out_proj, plus b_o/2); the host sums
the two partials per batch.

Layout strategy (all on-chip tensors "t-major", i.e. feature dim on
partitions, sequence on the free axis):
  xT      [dim=8x128, S]   via PE (tensor-engine) transposes of x
  kv_latT [128, S]         = w_kvc^T @ xT        (+b_kvc)
  q_latT  [256, S]         = w_qc^T @ xT         (+b_qc)
  KT      [512, S]         = w_kvu_k^T @ kv_latT (+b)    (local heads)
  QT      [512, S]         = w_qu^T   @ q_latT   (+b)
  V       [S, 520]         = kv_lat @ w_kvu_v    (+b), 65-col blocks per
                             head: 64 value cols + a ones column.
Attention per (s-half j, head pair), streaming over key chunks k:
  scoresT[t,s] via matmul (head pair shares the PE array via disjoint
  64-row groups), exp(s/8) on ScalarE, causal handled by clipping the
  s-range + affine_select on the diagonal block; PV accumulates
  ctx^T[64, s] in PSUM, the ones column gives the softmax denominator
  in row 64.  ctx scaled by 1/denom (reciprocal + partition-broadcast
  multiply) into ctxT, then out = ctxT^T @ w_o + b_o/2.

Matmul operands use float32r (single-pass fp32 streaming on the PE,
4x faster than exact fp32); producers write tiles with f32r dtype so
operands are pre-rounded.
"""

import numpy as np

import concourse.bass as bass
import concourse.bacc as bacc
import concourse.mybir as mybir
import concourse.tile as tile
from concourse import masks

DIM = 1024
NUM_HEADS = 16
HEAD_DIM = 64
LAT = 128
QR = 256
B = 4
NCORES = 8
ND = DIM // 128       # 8 d-chunks
NHL = 8               # heads per core
F32 = mybir.dt.float32
F32R = mybir.dt.float32r
AF = mybir.ActivationFunctionType


def _pieces(total, w=512):
    return [(o, min(w, total - o)) for o in range(0, total, w)]


def build_mla(S=2048, mmdt=F32R):
    """Build the per-core Bass program (same SPMD program on all 8 cores)."""
    assert S % 256 == 0
    SH = S // 2           # s-half width
    NT = S // 128         # number of 128-token chunks

    nc = bacc.Bacc()

    x_d = nc.declare_dram_parameter("x", [S, DIM], F32, isOutput=False)
    w_kvc_d = nc.declare_dram_parameter("w_kvc", [DIM, LAT], F32, isOutput=False)
    w_qc_d = nc.declare_dram_parameter("w_qc", [DIM, QR], F32, isOutput=False)
    w_kvu_k_d = nc.declare_dram_parameter("w_kvu_k", [LAT, 512], F32, isOutput=False)
    w_kvu_v_d = nc.declare_dram_parameter("w_kvu_v", [LAT, 512], F32, isOutput=False)
    w_qu_d = nc.declare_dram_parameter("w_qu", [QR, 512], F32, isOutput=False)
    w_o_d = nc.declare_dram_parameter("w_o", [512, DIM], F32, isOutput=False)
    b_kvc_d = nc.declare_dram_parameter("b_kvc", [LAT, 1], F32, isOutput=False)
    b_qc_d = nc.declare_dram_parameter("b_qc", [128, 2], F32, isOutput=False)
    b_qu_d = nc.declare_dram_parameter("b_qu", [128, 4], F32, isOutput=False)
    b_kvu_k_d = nc.declare_dram_parameter("b_kvu_k", [128, 4], F32, isOutput=False)
    b_kvu_v_d = nc.declare_dram_parameter("b_kvu_v", [1, 512], F32, isOutput=False)
    b_o_d = nc.declare_dram_parameter("b_o", [1, DIM], F32, isOutput=False)
    out_d = nc.declare_dram_parameter("out", [S, DIM], F32, isOutput=True)

    with tile.TileContext(nc) as tc:
        with (
            tc.tile_pool(name="const", bufs=1) as const,
            tc.tile_pool(name="wts", bufs=1) as wts,
            tc.tile_pool(name="big", bufs=1) as big,
            tc.tile_pool(name="stg", bufs=2) as stg,
        ):
            ident = const.tile([128, 128], F32, name="ident")
            masks.make_identity(nc, ident[:])
            # memset doesn't support f32r; memset f32 then round-copy
            ones1f = const.tile([1, 128], F32, name="ones1f")
            nc.gpsimd.memset(ones1f[:], 1.0)
            ones1 = const.tile([1, 128], mmdt, name="ones1")
            nc.vector.tensor_copy(ones1[:], ones1f[:])

            # ---- weights into SBUF (staged fp32 DMA, rounded copy to mmdt) --
            def load_rounded(dst_ap, src_ap, shape):
                st = stg.tile([128, 1024], F32, tag="stage")
                sap = st[:shape[0], :shape[1]]
                nc.sync.dma_start(out=sap, in_=src_ap)
                nc.vector.tensor_copy(dst_ap, sap)

            w_kvc_sb = wts.tile([128, DIM], mmdt, name="w_kvc_sb")
            w_qc_sb = wts.tile([128, ND * QR], mmdt, name="w_qc_sb")
            for dc in range(ND):
                load_rounded(w_kvc_sb[:, 128 * dc:128 * dc + 128],
                             w_kvc_d[128 * dc:128 * dc + 128, :], (128, 128))
                load_rounded(w_qc_sb[:, QR * dc:QR * dc + QR],
                             w_qc_d[128 * dc:128 * dc + 128, :], (128, QR))
            w_kvu_k_sb = wts.tile([128, 512], mmdt, name="w_kvu_k_sb")
            load_rounded(w_kvu_k_sb[:], w_kvu_k_d[:, :], (128, 512))
            w_kvu_v_sb = wts.tile([128, 512], mmdt, name="w_kvu_v_sb")
            load_rounded(w_kvu_v_sb[:], w_kvu_v_d[:, :], (128, 512))
            w_qu_sb = wts.tile([128, 1024], mmdt, name="w_qu_sb")
            for qc in range(2):
                load_rounded(w_qu_sb[:, 512 * qc:512 * qc + 512],
                             w_qu_d[128 * qc:128 * qc + 128, :], (128, 512))
            b_kvu_v_sb = wts.tile([1, 512], mmdt, name="b_kvu_v_sb")
            load_rounded(b_kvu_v_sb[:], b_kvu_v_d[:, :], (1, 512))
            b_o_sb = wts.tile([1, DIM], mmdt, name="b_o_sb")
            load_rounded(b_o_sb[:], b_o_d[:, :], (1, DIM))
            # preload w_o so phase E starts without waiting on its DMA
            w_o_sb = wts.tile([128, 4 * DIM], mmdt, name="w_o_sb")
            for cc in range(4):
                load_rounded(w_o_sb[:, DIM * cc:DIM * cc + DIM],
                             w_o_d[128 * cc:128 * cc + 128, :], (128, DIM))

            # per-partition bias vectors (not matmul operands -> plain f32)
            b_kvc_sb = wts.tile([128, 1], F32, name="b_kvc_sb")
            nc.sync.dma_start(out=b_kvc_sb[:], in_=b_kvc_d[:, :])
            b_qc_sb = wts.tile([128, 2], F32, name="b_qc_sb")
            nc.sync.dma_start(out=b_qc_sb[:], in_=b_qc_d[:, :])
            b_qu_sb = wts.tile([128, 4], F32, name="b_qu_sb")
            nc.sync.dma_start(out=b_qu_sb[:], in_=b_qu_d[:, :])
            b_kvu_k_sb = wts.tile([128, 4], F32, name="b_kvu_k_sb")
            nc.sync.dma_start(out=b_kvu_k_sb[:], in_=b_kvu_k_d[:, :])

            # ---- persistent products: KT / QT / V (chunk c lives at cols c*S) ----
            KT = big.tile([128, 4 * S], mmdt, name="KT")
            QT = big.tile([128, 4 * S], mmdt, name="QT")
            V = big.tile([128, NT * 520], mmdt, name="V")
            # ones columns of V (col 64 of each 65-wide head block);
            # memset doesn't support f32r, so copy from an f32 ones tile
            v_view = V[:].rearrange("p (k h c) -> p k h c", h=NHL, c=65)
            ones_cols = const.tile([128, NT * NHL], F32, name="ones_cols")
            nc.gpsimd.memset(ones_cols[:], 1.0)
            nc.vector.tensor_copy(
                v_view[:, :, :, 64:65],
                ones_cols[:].rearrange("p (k h o) -> p k h o", h=NHL, o=1))

            # ================= phase A+B+C: transpose + projections =========
            with (
                tc.tile_pool(name="xin", bufs=3) as xin,
                tc.tile_pool(name="xtp", bufs=2) as xtp,
                tc.tile_pool(name="kvq", bufs=2) as kvq,
                tc.tile_pool(name="tpps", bufs=1, space="PSUM") as tpps,
                tc.tile_pool(name="pjps", bufs=1, space="PSUM") as pjps,
            ):
                for off, w in _pieces(S):
                    ntile = w // 128
                    # transpose x rows [off, off+w) -> xTp [128, 8 * w]
                    # (d-chunk dc at cols dc*w)
                    xTp = xtp.tile([128, ND * 512], mmdt, tag="xTp")
                    for q in range(ntile):
                        xt = xin.tile([128, DIM], F32, tag="xin")
                        nc.sync.dma_start(
                            out=xt[:],
                            in_=x_d[off + 128 * q:off + 128 * q + 128, :])
                        for dg in range(2):
                            ps = tpps.tile([128, 512], F32, tag="tp", bufs=2)
                            for u in range(4):
                                dc = 4 * dg + u
                                nc.tensor.transpose(
                                    ps[:, 128 * u:128 * u + 128],
                                    xt[:, 128 * dc:128 * dc + 128],
                                    ident[:])
                            dst = xTp[:].rearrange(
                                "p (d t) -> p d t", t=512
                            )[:, 4 * dg:4 * dg + 4, 128 * q:128 * q + 128]
                            src = ps[:].rearrange("p (d t) -> p d t", t=128)
                            nc.vector.tensor_copy(dst, src)
                    # kv_lat / q_lat for this piece
                    kvp = pjps.tile([128, 512], F32, tag="kv", bufs=1)
                    q0p = pjps.tile([128, 512], F32, tag="q0", bufs=1)
                    q1p = pjps.tile([128, 512], F32, tag="q1", bufs=1)
                    for dc in range(ND):
                        xr = xTp[:, dc * 512:dc * 512 + w]
                        st = dc == 0
                        sp = dc == ND - 1
                        nc.tensor.matmul(
                            kvp[:, :w], w_kvc_sb[:, 128 * dc:128 * dc + 128],
                            xr, start=st, stop=sp)
                        nc.tensor.matmul(
                            q0p[:, :w], w_qc_sb[:, QR * dc:QR * dc + 128],
                            xr, start=st, stop=sp)
                        nc.tensor.matmul(
                            q1p[:, :w], w_qc_sb[:, QR * dc + 128:QR * dc + 256],
                            xr, start=st, stop=sp)
                    kvs = kvq.tile([128, 512], mmdt, tag="kvs")
                    q0s = kvq.tile([128, 512], mmdt, tag="q0s")
                    q1s = kvq.tile([128, 512], mmdt, tag="q1s")
                    nc.vector.tensor_scalar_add(kvs[:, :w], kvp[:, :w], b_kvc_sb[:, 0:1])
                    nc.vector.tensor_scalar_add(q0s[:, :w], q0p[:, :w], b_qc_sb[:, 0:1])
                    nc.vector.tensor_scalar_add(q1s[:, :w], q1p[:, :w], b_qc_sb[:, 1:2])
                    # K^T / Q^T chunks for this piece
                    for c in range(4):
                        kp = pjps.tile([128, 512], F32, tag="pjo", bufs=2)
                        nc.tensor.matmul(
                            kp[:, :w], w_kvu_k_sb[:, 128 * c:128 * c + 128],
                            kvs[:, :w], start=True, stop=True)
                        nc.vector.tensor_scalar_add(
                            KT[:, c * S + off:c * S + off + w], kp[:, :w],
                            b_kvu_k_sb[:, c:c + 1])
                        qp = pjps.tile([128, 512], F32, tag="pjo", bufs=2)
                        nc.tensor.matmul(
                            qp[:, :w], w_qu_sb[:, 128 * c:128 * c + 128],
                            q0s[:, :w], start=True, stop=False)
                        nc.tensor.matmul(
                            qp[:, :w], w_qu_sb[:, 512 + 128 * c:512 + 128 * c + 128],
                            q1s[:, :w], start=False, stop=True)
                        nc.vector.tensor_scalar_add(
                            QT[:, c * S + off:c * S + off + w], qp[:, :w],
                            b_qu_sb[:, c:c + 1])
                    # V chunks for this piece
                    for q in range(ntile):
                        k = (off + 128 * q) // 128
                        vp = pjps.tile([128, 512], F32, tag="pjo", bufs=2)
                        nc.tensor.matmul(vp[:], ones1[0:1, :], b_kvu_v_sb[0:1, :],
                                         start=True, stop=False)
                        nc.tensor.matmul(vp[:], kvs[:, 128 * q:128 * q + 128],
                                         w_kvu_v_sb[:], start=False, stop=True)
                        nc.vector.tensor_copy(
                            v_view[:, k, :, 0:64],
                            vp[:].rearrange("p (h c) -> p h c", c=64))

            # ================= phase D: attention ===========================
            with tc.tile_pool(name="ctxTp", bufs=1) as ctxTp:
                ctxT = ctxTp.tile([128, 4 * S], mmdt, name="ctxT")
                with (
                    tc.tile_pool(name="attn", bufs=1) as attn,
                    tc.tile_pool(name="scps", bufs=1, space="PSUM") as scps,
                    tc.tile_pool(name="ctxps", bufs=2, space="PSUM") as ctxps,
                ):
                    nbank = (SH + 511) // 512
                    for j in range(2):
                        s0 = SH * j
                        kmax = (SH // 128) * (j + 1)
                        last_k = {
                            bi: min(kmax - 1, (s0 + 512 * (bi + 1)) // 128 - 1)
                            for bi in range(nbank)
                        }
                        for hp in range(NHL // 2):
                            heads = (2 * hp, 2 * hp + 1)
                            c = hp // 1  # KT/QT chunk = hp
                            ctxs = [ctxps.tile([65, SH], F32, tag="ctx",
                                               name=f"ctx{h}") for h in heads]
                            for k in range(kmax):
                                t0 = 128 * k
                                ss = max(s0, t0)
                                fd = s0 + SH - ss
                                rel = ss - s0
                                scs = []
                                # the two heads' QK matmuls are adjacent and
                                # use disjoint 64-row groups of the PE array
                                for o2, w2 in _pieces(fd):
                                    for hi, h in enumerate(heads):
                                        po = 64 * (h % 2)
                                        if o2 == 0:
                                            scs.append(scps.tile(
                                                [128, SH], F32, tag="sc",
                                                bufs=2, name=f"sc{h}"))
                                        nc.tensor.matmul(
                                            scs[hi][:, o2:o2 + w2],
                                            KT[po:po + 64,
                                               hp * S + t0:hp * S + t0 + 128],
                                            QT[po:po + 64,
                                               hp * S + ss + o2:hp * S + ss + o2 + w2],
                                            start=True, stop=True)
                                exs = []
                                for hi, h in enumerate(heads):
                                    ex = attn.tile([128, SH], mmdt, tag="ex",
                                                   bufs=4, name=f"ex{h}")
                                    exs.append(ex)
                                    nc.scalar.activation(ex[:, :fd], scs[hi][:, :fd],
                                                         AF.Exp, scale=0.125)
                                    if t0 >= s0:
                                        nc.gpsimd.affine_select(
                                            out=ex[:, 0:128], in_=ex[:, 0:128],
                                            pattern=[[1, 128]],
                                            compare_op=mybir.AluOpType.is_ge,
                                            fill=0.0, base=0, channel_multiplier=-1)
                                for hi, h in enumerate(heads):
                                    for bi in range(nbank):
                                        a = max(rel, 512 * bi)
                                        b2 = min(SH, 512 * bi + 512)
                                        if a >= b2:
                                            continue
                                        nc.tensor.matmul(
                                            ctxs[hi][:, a:b2],
                                            V[:, 520 * k + 65 * h:520 * k + 65 * h + 65],
                                            exs[hi][:, a - rel:b2 - rel],
                                            start=(k == 0), stop=(k == last_k[bi]))
                            # normalize: ctx[0:64] * (1/ctx[64])
                            for hi, h in enumerate(heads):
                                po = 64 * (h % 2)
                                rec = attn.tile([1, SH], F32, tag="rec", bufs=1,
                                                name=f"rec{h}")
                                nc.vector.reciprocal(rec[:], ctxs[hi][64:65, :])
                                rbc = attn.tile([64, SH], F32, tag="rbc", bufs=1,
                                                name=f"rbc{h}")
                                nc.gpsimd.partition_broadcast(rbc[:], rec[0:1, :])
                                nc.vector.tensor_mul(
                                    ctxT[po:po + 64, hp * S + s0:hp * S + s0 + SH],
                                    ctxs[hi][0:64, :], rbc[:])

            # ================= phase E: out projection ======================
                with (
                    tc.tile_pool(name="outsb", bufs=3) as outsb,
                    tc.tile_pool(name="ops", bufs=2, space="PSUM") as ops,
                ):
                    for si in range(NT):
                        op = ops.tile([128, DIM], F32, tag="op")
                        for o2, w2 in _pieces(DIM):
                            nc.tensor.matmul(op[:, o2:o2 + w2], ones1[0:1, :],
                                             b_o_sb[0:1, o2:o2 + w2],
                                             start=True, stop=False)
                        for cc in range(4):
                            for o2, w2 in _pieces(DIM):
                                nc.tensor.matmul(
                                    op[:, o2:o2 + w2],
                                    ctxT[:, cc * S + 128 * si:cc * S + 128 * si + 128],
                                    w_o_sb[:, DIM * cc + o2:DIM * cc + o2 + w2],
                                    start=False, stop=(cc == 3))
                        ob = outsb.tile([128, DIM], F32, tag="ob")
                        nc.vector.tensor_copy(ob[:, 0:512], op[:, 0:512])
                        nc.scalar.copy(ob[:, 512:DIM], op[:, 512:DIM])
                        nc.sync.dma_start(
                            out=out_d[128 * si:128 * si + 128, :], in_=ob[:])

    nc.finalize()
    return nc


def shard_inputs(inputs, S=2048):
    """Build the 8 per-core input maps from full inputs."""
    f = lambda a: np.ascontiguousarray(np.asarray(a, dtype=np.float32))
    x = f(inputs["x"])
    w_kvc, b_kvc = f(inputs["w_kvc"]), f(inputs["b_kvc"])
    w_kvu, b_kvu = f(inputs["w_kvu"]), f(inputs["b_kvu"])
    w_qc, b_qc = f(inputs["w_qc"]), f(inputs["b_qc"])
    w_qu, b_qu = f(inputs["w_qu"]), f(inputs["b_qu"])
    w_o, b_o = f(inputs["w_o"]), f(inputs["b_o"])
    in_maps = []
    for core in range(NCORES):
        b = core // 2
        g = core % 2
        cs = slice(512 * g, 512 * g + 512)
        in_maps.append({
            "x": x[b],
            "w_kvc": w_kvc,
            "w_qc": w_qc,
            "w_kvu_k": np.ascontiguousarray(w_kvu[:, 512 * g:512 * g + 512]),
            "w_kvu_v": np.ascontiguousarray(w_kvu[:, 1024 + 512 * g:1024 + 512 * g + 512]),
            "w_qu": np.ascontiguousarray(w_qu[:, cs]),
            "w_o": np.ascontiguousarray(w_o[cs, :]),
            "b_kvc": b_kvc.reshape(LAT, 1),
            "b_qc": np.ascontiguousarray(b_qc.reshape(2, 128).T),
            "b_qu": np.ascontiguousarray(b_qu[cs].reshape(4, 128).T),
            "b_kvu_k": np.ascontiguousarray(b_kvu[cs].reshape(4, 128).T),
            "b_kvu_v": np.ascontiguousarray(b_kvu[1024 + 512 * g:1024 + 512 * g + 512].reshape(1, 512)),
            "b_o": np.ascontiguousarray((b_o * 0.5).reshape(1, DIM)),
        })
    return in_maps


def kernel(**inputs) -> np.ndarray:
    from concourse.bass_utils import run_bass_kernel_spmd

    x = np.asarray(inputs["x"])
    S = x.shape[1]
    nc = build_mla(S=S)
    in_maps = shard_inputs(inputs, S=S)
    res = run_bass_kernel_spmd(nc, in_maps, list(range(NCORES))).results
    out = np.empty((B, S, DIM), dtype=np.float32)
    for b in range(B):
        out[b] = res[2 * b]["out"] + res[2 * b + 1]["out"]
    return out



# revision 7
# speedup vs baseline: 1.1928x; 1.1928x over previous
"""MLA (multi-head latent attention) Bass kernel for Trainium2, 8 NeuronCores.

Sharding: core i handles batch b = i // 2 and head-group g = i % 2
(8 of the 16 heads).  Each core computes a partial output
(its heads' contribution through out_proj, plus b_o/2); the host sums
the two partials per batch.

v2 design notes (all driven by the CoreSim v1 cost model):
  - The host pre-transposes x to [dim, S] and pre-casts x + all weights
    to bf16, so there are no on-chip PE transposes and no weight staging
    copies.  bf16 matmuls are 1 cycle/row at any moving width (f32r
    degrades 4x below 256), and halve SBUF/DMA traffic.
  - Row-vector biases (b_kvu_v, b_o) arrive pre-broadcast as [128, n]
    tiles and are folded into the PSUM->SBUF drain via tensor_add on the
    DVE, so the PE never runs bias matmuls.  Per-partition biases
    (b_kvc/b_qc/b_qu/b_kvu_k) fold into tensor_scalar_add drains.
  - Layout: everything "t-major" (feature dim on partitions):
      xT [8x128, S] (DMA'd), kv_latT [128,S], q_latT [256,S],
      KT/QT [128 = 2 heads x 64, 4 chunks x S] bf16,
      V [128 tokens, 16 chunks x (8 heads x 65)] bf16 with a ones
      column per head (col 64) that accumulates the softmax denominator
      during the PV matmul (row 64 of ctx PSUM).
  - Attention per (s-half j, head h): stream key chunks k; QK scores to
    PSUM, exp(s/8) on the scalar engine (the only Activation-engine
    work in the kernel), causal diagonal via gpsimd affine_select, PV
    accumulates ctx^T[65, SH] in PSUM; normalize via DVE reciprocal +
    gpsimd partition_broadcast + DVE multiply into ctxT bf16.
  - Software pipelining by emission order: a minimal projection prefix,
    then attention j=0 interleaved with the remaining projection work,
    then attention j=1 interleaved with out_proj of the first half,
    then the out_proj tail.  Interleaved "fill" work keeps the PE busy
    while the scalar engine (the attention-phase bottleneck) runs exp.
  - PSUM budget (8 banks): sc slots 2x[128,1024]f32 = 4, ctx
    [65,1024]f32 = 2, shared misc pool [128,512]f32 x2 = 2.
"""

import numpy as np

import concourse.bass as bass
import concourse.bacc as bacc
import concourse.mybir as mybir
import concourse.tile as tile

DIM = 1024
NUM_HEADS = 16
HEAD_DIM = 64
LAT = 128
QR = 256
B = 4
NCORES = 8
ND = DIM // 128       # 8 d-chunks
NHL = 8               # heads per core
F32 = mybir.dt.float32
BF16 = mybir.dt.bfloat16
AF = mybir.ActivationFunctionType


def _pieces(total, w=512):
    return [(o, min(w, total - o)) for o in range(0, total, w)]


def build_mla(S=2048):
    """Build the per-core Bass program (same SPMD program on all 8 cores)."""
    assert S % 1024 == 0
    SH = S // 2           # s-half width
    NT = S // 128         # number of 128-token chunks
    NP = S // 512         # number of 512-token projection pieces

    nc = bacc.Bacc()

    xT_d = nc.declare_dram_parameter("xT", [DIM, S], BF16, isOutput=False)
    w_kvc_d = nc.declare_dram_parameter("w_kvc", [DIM, LAT], BF16, isOutput=False)
    w_qc_d = nc.declare_dram_parameter("w_qc", [DIM, QR], BF16, isOutput=False)
    w_kvu_k_d = nc.declare_dram_parameter("w_kvu_k", [LAT, 512], BF16, isOutput=False)
    w_kvu_v_d = nc.declare_dram_parameter("w_kvu_v", [LAT, 512], BF16, isOutput=False)
    w_qu_d = nc.declare_dram_parameter("w_qu", [QR, 512], BF16, isOutput=False)
    w_o_d = nc.declare_dram_parameter("w_o", [512, DIM], BF16, isOutput=False)
    b_kvc_d = nc.declare_dram_parameter("b_kvc", [LAT, 1], F32, isOutput=False)
    b_qc_d = nc.declare_dram_parameter("b_qc", [128, 2], F32, isOutput=False)
    b_qu_d = nc.declare_dram_parameter("b_qu", [128, 4], F32, isOutput=False)
    b_kvu_k_d = nc.declare_dram_parameter("b_kvu_k", [128, 4], F32, isOutput=False)
    b_kvu_v_d = nc.declare_dram_parameter("b_kvu_v", [128, 512], F32, isOutput=False)
    b_o_d = nc.declare_dram_parameter("b_o", [128, DIM], F32, isOutput=False)
    out_d = nc.declare_dram_parameter("out", [S, DIM], F32, isOutput=True)

    with tile.TileContext(nc) as tc:
        with (
            tc.tile_pool(name="const", bufs=1) as const,
            tc.tile_pool(name="wts", bufs=1) as wts,
            tc.tile_pool(name="big", bufs=1) as big,
            tc.tile_pool(name="xin", bufs=4) as xin,
            tc.tile_pool(name="kvq", bufs=2) as kvq,
            tc.tile_pool(name="exd", bufs=4) as exd,
            tc.tile_pool(name="nrm", bufs=2) as nrm,
            tc.tile_pool(name="outs", bufs=3) as outs,
            tc.tile_pool(name="mps", bufs=2, space="PSUM") as mps,
        ):
            # ---- weight tiles (DMA'd inside W1, after the x DMAs) ----------
            w_kvc_sb = wts.tile([128, DIM], BF16, name="w_kvc_sb")
            w_qc_sb = wts.tile([128, ND * QR], BF16, name="w_qc_sb")
            w_kvu_k_sb = wts.tile([128, 512], BF16, name="w_kvu_k_sb")
            w_kvu_v_sb = wts.tile([128, 512], BF16, name="w_kvu_v_sb")
            w_qu_sb = wts.tile([128, 1024], BF16, name="w_qu_sb")
            w_o_sb = wts.tile([128, 4 * DIM], BF16, name="w_o_sb")
            b_kvc_sb = wts.tile([128, 1], F32, name="b_kvc_sb")
            b_qc_sb = wts.tile([128, 2], F32, name="b_qc_sb")
            b_qu_sb = wts.tile([128, 4], F32, name="b_qu_sb")
            b_kvu_k_sb = wts.tile([128, 4], F32, name="b_kvu_k_sb")
            b_kvu_v_sb = wts.tile([128, 512], F32, name="b_kvu_v_sb")
            b_o_sb = wts.tile([128, DIM], F32, name="b_o_sb")

            def emit_wdma():
                # critical-path first: latent weights, then up-proj, then
                # V/out-proj weights needed later.
                for dc in range(ND):
                    nc.sync.dma_start(
                        out=w_kvc_sb[:, 128 * dc:128 * dc + 128],
                        in_=w_kvc_d[128 * dc:128 * dc + 128, :])
                    nc.sync.dma_start(
                        out=w_qc_sb[:, QR * dc:QR * dc + QR],
                        in_=w_qc_d[128 * dc:128 * dc + 128, :])
                nc.sync.dma_start(out=b_kvc_sb[:], in_=b_kvc_d[:, :])
                nc.sync.dma_start(out=b_qc_sb[:], in_=b_qc_d[:, :])
                nc.sync.dma_start(out=w_kvu_k_sb[:], in_=w_kvu_k_d[:, :])
                for qc in range(2):
                    nc.sync.dma_start(
                        out=w_qu_sb[:, 512 * qc:512 * qc + 512],
                        in_=w_qu_d[128 * qc:128 * qc + 128, :])
                nc.sync.dma_start(out=b_qu_sb[:], in_=b_qu_d[:, :])
                nc.sync.dma_start(out=b_kvu_k_sb[:], in_=b_kvu_k_d[:, :])
                nc.sync.dma_start(out=w_kvu_v_sb[:], in_=w_kvu_v_d[:, :])
                nc.sync.dma_start(out=b_kvu_v_sb[:], in_=b_kvu_v_d[:, :])
                for cc in range(4):
                    nc.sync.dma_start(
                        out=w_o_sb[:, DIM * cc:DIM * cc + DIM],
                        in_=w_o_d[128 * cc:128 * cc + 128, :])
                nc.sync.dma_start(out=b_o_sb[:], in_=b_o_d[:, :])

            # ---- persistent products ---------------------------------------
            KT = big.tile([128, 4 * S], BF16, name="KT")
            QT = big.tile([128, 4 * S], BF16, name="QT")
            V = big.tile([128, NT * 520], BF16, name="V")
            ctxT = big.tile([128, 4 * S], BF16, name="ctxT")
            v_view = V[:].rearrange("p (k h c) -> p k h c", h=NHL, c=65)
            # ones columns of V (col 64 of each 65-wide head block)
            ones_f = const.tile([128, NT * NHL], F32, name="ones_f")
            nc.gpsimd.memset(ones_f[:], 1.0)
            nc.vector.tensor_copy(
                v_view[:, :, :, 64:65],
                ones_f[:].rearrange("p (k h o) -> p k h o", h=NHL, o=1))

            # ---- projection emitters ---------------------------------------
            xts = [None] * NP
            kvs = [None] * NP
            q0s = [None] * NP
            q1s = [None] * NP

            def emit_xdma(p):
                xt = xin.tile([128, ND * 512], BF16, tag="xt")
                xts[p] = xt
                for dc in range(ND):
                    nc.sync.dma_start(
                        out=xt[:, 512 * dc:512 * dc + 512],
                        in_=xT_d[128 * dc:128 * dc + 128, 512 * p:512 * p + 512])

            def emit_lat_kv(p, pool):
                ps = pool.tile([128, 512], F32, tag="mm")
                for dc in range(ND):
                    nc.tensor.matmul(
                        ps[:], w_kvc_sb[:, 128 * dc:128 * dc + 128],
                        xts[p][:, 512 * dc:512 * dc + 512],
                        start=dc == 0, stop=dc == ND - 1)
                t = kvq.tile([128, 512], BF16, tag="kvs")
                kvs[p] = t
                nc.vector.tensor_scalar_add(t[:], ps[:], b_kvc_sb[:, 0:1])

            def emit_lat_q(p, half, pool):
                ps = pool.tile([128, 512], F32, tag="mm")
                for dc in range(ND):
                    nc.tensor.matmul(
                        ps[:], w_qc_sb[:, QR * dc + 128 * half:QR * dc + 128 * half + 128],
                        xts[p][:, 512 * dc:512 * dc + 512],
                        start=dc == 0, stop=dc == ND - 1)
                t = kvq.tile([128, 512], BF16, tag=f"q{half}s")
                (q0s if half == 0 else q1s)[p] = t
                nc.vector.tensor_scalar_add(t[:], ps[:], b_qc_sb[:, half:half + 1])

            def emit_qt(p, c, pool):
                ps = pool.tile([128, 512], F32, tag="mm")
                nc.tensor.matmul(ps[:], w_qu_sb[:, 128 * c:128 * c + 128],
                                 q0s[p][:], start=True, stop=False)
                nc.tensor.matmul(ps[:], w_qu_sb[:, 512 + 128 * c:512 + 128 * c + 128],
                                 q1s[p][:], start=False, stop=True)
                nc.vector.tensor_scalar_add(
                    QT[:, c * S + 512 * p:c * S + 512 * p + 512], ps[:],
                    b_qu_sb[:, c:c + 1])

            def emit_kt(p, c, pool):
                ps = pool.tile([128, 512], F32, tag="mm")
                nc.tensor.matmul(ps[:], w_kvu_k_sb[:, 128 * c:128 * c + 128],
                                 kvs[p][:], start=True, stop=True)
                nc.vector.tensor_scalar_add(
                    KT[:, c * S + 512 * p:c * S + 512 * p + 512], ps[:],
                    b_kvu_k_sb[:, c:c + 1])

            def emit_v(p, q, pool):
                k = 4 * p + q
                ps = pool.tile([128, 512], F32, tag="mm")
                nc.tensor.matmul(ps[:], kvs[p][:, 128 * q:128 * q + 128],
                                 w_kvu_v_sb[:], start=True, stop=True)
                nc.vector.tensor_add(
                    v_view[:, k, :, 0:64],
                    ps[:].rearrange("p (h c) -> p h c", c=64),
                    b_kvu_v_sb[:].rearrange("p (h c) -> p h c", c=64))

            def emit_outproj_half(si, o2, pool):
                ps = pool.tile([128, 512], F32, tag="mm")
                for cc in range(4):
                    nc.tensor.matmul(
                        ps[:],
                        ctxT[:, cc * S + 128 * si:cc * S + 128 * si + 128],
                        w_o_sb[:, DIM * cc + o2:DIM * cc + o2 + 512],
                        start=cc == 0, stop=cc == 3)
                ob = obs[si % 4]
                nc.vector.tensor_add(ob[:, o2:o2 + 512], ps[:],
                                     b_o_sb[:, o2:o2 + 512])
                if o2 == 512:
                    nc.sync.dma_start(
                        out=out_d[128 * si:128 * si + 128, :], in_=ob[:])

            obs = [None] * 4

            def outproj_steps(si_range):
                for si in si_range:
                    obs[si % 4] = outs.tile([128, DIM], F32, tag="ob",
                                            name=f"ob{si}")
                    yield lambda pool, si=si: emit_outproj_half(si, 0, pool)
                    yield lambda pool, si=si: emit_outproj_half(si, 512, pool)

            # ---- attention ------------------------------------------------
            def attn_head(j, h, scp, ctp, fill):
                """Attention for s-half j, local head h; calls fill() between
                emitted steps to interleave projection/out_proj PE work."""
                s0 = SH * j
                c = h // 2
                po = 64 * (h % 2)
                kmax = (SH // 128) * (j + 1)
                nbank = SH // 512
                last_k = {bi: min(kmax - 1, (s0 + 512 * (bi + 1)) // 128 - 1)
                          for bi in range(nbank)}
                ctx = ctp.tile([65, SH], F32, tag="ctx")
                for k in range(kmax):
                    t0 = 128 * k
                    ss = max(s0, t0)
                    fd = s0 + SH - ss
                    rel = ss - s0
                    sc = scp.tile([128, SH], F32, tag="sc")
                    for o2, w2 in _pieces(fd):
                        nc.tensor.matmul(
                            sc[:, o2:o2 + w2],
                            KT[po:po + 64, c * S + t0:c * S + t0 + 128],
                            QT[po:po + 64, c * S + ss + o2:c * S + ss + o2 + w2],
                            start=True, stop=True)
                    ex = exd.tile([128, SH], BF16, tag="ex")
                    nc.scalar.activation(ex[:, :fd], sc[:, :fd], AF.Exp,
                                         scale=0.125)
                    if t0 >= s0:
                        nc.gpsimd.affine_select(
                            out=ex[:, 0:128], in_=ex[:, 0:128],
                            pattern=[[1, 128]],
                            compare_op=mybir.AluOpType.is_ge,
                            fill=0.0, base=0, channel_multiplier=-1)
                    fill()
                    for bi in range(nbank):
                        a = max(rel, 512 * bi)
                        b2 = min(SH, 512 * bi + 512)
                        if a >= b2:
                            continue
                        nc.tensor.matmul(
                            ctx[:, a:b2],
                            V[:, 520 * k + 65 * h:520 * k + 65 * h + 65],
                            ex[:, a - rel:b2 - rel],
                            start=(k == 0), stop=(k == last_k[bi]))
                # normalize: ctx[0:64] * (1/ctx[64]) -> ctxT
                rec = nrm.tile([1, SH], F32, tag="rec")
                nc.vector.reciprocal(rec[:], ctx[64:65, :])
                rbc = nrm.tile([64, SH], F32, tag="rbc")
                nc.gpsimd.partition_broadcast(rbc[:], rec[0:1, :])
                nc.vector.tensor_mul(
                    ctxT[po:po + 64, c * S + s0:c * S + s0 + SH],
                    ctx[0:64, :], rbc[:])

            # ================= W1: minimal projection prefix =================
            with tc.tile_pool(name="w1p", bufs=4, space="PSUM") as w1p:
                emit_xdma(0)
                emit_wdma()
                emit_xdma(1)
                emit_xdma(2)
                emit_xdma(3)
                for p in (0, 1):
                    emit_lat_kv(p, w1p)
                    emit_lat_q(p, 0, w1p)
                    emit_lat_q(p, 1, w1p)
                emit_qt(0, 0, w1p)
                emit_qt(1, 0, w1p)
                emit_kt(0, 0, w1p)
                emit_kt(1, 0, w1p)
                for q in range(4):
                    emit_v(0, q, w1p)

            # fill steps for the j=0 attention window, in dependency order.
            # head h needs KT/QT chunk c=h//2 for pieces 0,1 and V chunks 0-7.
            w2_fills = []
            for q in range(4):
                w2_fills.append(("v01", lambda pool, q=q: emit_v(1, q, pool)))
            for c in (1, 2, 3):
                for p in (0, 1):
                    w2_fills.append((f"kq{c}",
                                     lambda pool, p=p, c=c: emit_qt(p, c, pool)))
                    w2_fills.append((f"kq{c}",
                                     lambda pool, p=p, c=c: emit_kt(p, c, pool)))
            for p in (2, 3):
                w2_fills.append(("p23", lambda pool, p=p: emit_lat_kv(p, pool)))
                w2_fills.append(("p23", lambda pool, p=p: emit_lat_q(p, 0, pool)))
                w2_fills.append(("p23", lambda pool, p=p: emit_lat_q(p, 1, pool)))
                for c in range(4):
                    w2_fills.append(("p23", lambda pool, p=p, c=c: emit_qt(p, c, pool)))
                    w2_fills.append(("p23", lambda pool, p=p, c=c: emit_kt(p, c, pool)))
                for q in range(4):
                    w2_fills.append(("p23", lambda pool, p=p, q=q: emit_v(p, q, pool)))

            fill_pos = [0]

            def drain(n):
                while n > 0 and fill_pos[0] < len(w2_fills):
                    w2_fills[fill_pos[0]][1](mps)
                    fill_pos[0] += 1
                    n -= 1

            def drain_until(label):
                idx = max((i for i, (lb, _) in enumerate(w2_fills) if lb == label),
                          default=-1)
                while fill_pos[0] <= idx:
                    w2_fills[fill_pos[0]][1](mps)
                    fill_pos[0] += 1

            with (
                tc.tile_pool(name="scp", bufs=2, space="PSUM") as scp,
                tc.tile_pool(name="ctp", bufs=1, space="PSUM") as ctp,
            ):
                # ================= W2: attention j=0 + fills ================
                for h in range(NHL):
                    if h >= 2:
                        drain_until(f"kq{h // 2}")
                    if h == 1:
                        drain_until("v01")
                    attn_head(0, h, scp, ctp, lambda: drain(1))
                drain(len(w2_fills))

                # ================= W3: attention j=1 + out_proj j0 ==========
                w3 = outproj_steps(range(8))
                w3_done = [False]

                def drain3():
                    if not w3_done[0]:
                        try:
                            next(w3)(mps)
                        except StopIteration:
                            w3_done[0] = True

                for h in range(NHL):
                    attn_head(1, h, scp, ctp, drain3)
                while not w3_done[0]:
                    drain3()

            # ================= W4: out_proj j1 tail =========================
            for step in outproj_steps(range(8, 16)):
                step(mps)

    nc.finalize()
    return nc


def shard_inputs(inputs, S=2048):
    """Build the 8 per-core input maps from full inputs (host-side prep:
    transpose x, cast matmul operands to bf16, pre-broadcast row biases)."""
    import ml_dtypes
    bf = lambda a: np.ascontiguousarray(np.asarray(a)).astype(ml_dtypes.bfloat16)
    f = lambda a: np.ascontiguousarray(np.asarray(a, dtype=np.float32))
    x = np.asarray(inputs["x"], dtype=np.float32)
    w_kvc, b_kvc = inputs["w_kvc"], f(inputs["b_kvc"])
    w_kvu, b_kvu = np.asarray(inputs["w_kvu"]), f(inputs["b_kvu"])
    w_qc, b_qc = inputs["w_qc"], f(inputs["b_qc"])
    w_qu, b_qu = np.asarray(inputs["w_qu"]), f(inputs["b_qu"])
    w_o, b_o = np.asarray(inputs["w_o"]), f(inputs["b_o"])
    xT = [bf(x[b].T) for b in range(B)]
    w_kvc_b = bf(w_kvc)
    w_qc_b = bf(w_qc)
    in_maps = []
    for core in range(NCORES):
        b = core // 2
        g = core % 2
        cs = slice(512 * g, 512 * g + 512)
        in_maps.append({
            "xT": xT[b],
            "w_kvc": w_kvc_b,
            "w_qc": w_qc_b,
            "w_kvu_k": bf(w_kvu[:, cs]),
            "w_kvu_v": bf(w_kvu[:, 1024 + 512 * g:1024 + 512 * g + 512]),
            "w_qu": bf(w_qu[:, cs]),
            "w_o": bf(w_o[cs, :]),
            "b_kvc": b_kvc.reshape(LAT, 1),
            "b_qc": np.ascontiguousarray(b_qc.reshape(2, 128).T),
            "b_qu": np.ascontiguousarray(b_qu[cs].reshape(4, 128).T),
            "b_kvu_k": np.ascontiguousarray(b_kvu[cs].reshape(4, 128).T),
            "b_kvu_v": np.ascontiguousarray(np.broadcast_to(
                b_kvu[1024 + 512 * g:1024 + 512 * g + 512], (128, 512))),
            "b_o": np.ascontiguousarray(np.broadcast_to(b_o * 0.5, (128, DIM))),
        })
    return in_maps


def kernel(**inputs) -> np.ndarray:
    from concourse.bass_utils import run_bass_kernel_spmd

    x = np.asarray(inputs["x"])
    S = x.shape[1]
    nc = build_mla(S=S)
    in_maps = shard_inputs(inputs, S=S)
    res = run_bass_kernel_spmd(nc, in_maps, list(range(NCORES))).results
    out = np.empty((B, S, DIM), dtype=np.float32)
    for b in range(B):
        out[b] = res[2 * b]["out"] + res[2 * b + 1]["out"]
    return out


# revision 14
# speedup vs baseline: 1.2721x; 1.0665x over previous
"""MLA (multi-head latent attention) Bass kernel for Trainium2, 8 NeuronCores.

Sharding: core i handles batch b = i // 2 and head-group g = i % 2
(8 of the 16 heads).  Each core computes a partial output
(its heads' contribution through out_proj, plus b_o/2); the host sums
the two partials per batch.

v2 design notes (all driven by the CoreSim v1 cost model):
  - The host pre-transposes x to [dim, S] and pre-casts x + all weights
    to bf16, so there are no on-chip PE transposes and no weight staging
    copies.  bf16 matmuls are 1 cycle/row at any moving width (f32r
    degrades 4x below 256), and halve SBUF/DMA traffic.
  - Row-vector biases (b_kvu_v, b_o) arrive pre-broadcast as [128, n]
    tiles and are folded into the PSUM->SBUF drain via tensor_add on the
    DVE, so the PE never runs bias matmuls.  Per-partition biases
    (b_kvc/b_qc/b_qu/b_kvu_k) fold into tensor_scalar_add drains.
  - Layout: everything "t-major" (feature dim on partitions):
      xT [8x128, S] (DMA'd), kv_latT [128,S], q_latT [256,S],
      KT/QT [128 = 2 heads x 64, 4 chunks x S] bf16,
      V [128 tokens, 16 chunks x (8 heads x 65)] bf16 with a ones
      column per head (col 64) that accumulates the softmax denominator
      during the PV matmul (row 64 of ctx PSUM).
  - Attention per (s-half j, head h): stream key chunks k; QK scores to
    PSUM, exp(s/8) on the scalar engine (the only Activation-engine
    work in the kernel), causal diagonal via gpsimd affine_select, PV
    accumulates ctx^T[65, SH] in PSUM; normalize via DVE reciprocal +
    gpsimd partition_broadcast + DVE multiply into ctxT bf16.
  - Software pipelining by emission order: a minimal projection prefix,
    then attention j=0 interleaved with the remaining projection work,
    then attention j=1 interleaved with out_proj of the first half,
    then the out_proj tail.  Interleaved "fill" work keeps the PE busy
    while the scalar engine (the attention-phase bottleneck) runs exp.
  - PSUM budget (8 banks): sc slots 2x[128,1024]f32 = 4, ctx
    [65,1024]f32 = 2, shared misc pool [128,512]f32 x2 = 2.
"""

import numpy as np

import concourse.bass as bass
import concourse.bacc as bacc
import concourse.mybir as mybir
import concourse.tile as tile

DIM = 1024
NUM_HEADS = 16
HEAD_DIM = 64
LAT = 128
QR = 256
B = 4
NCORES = 8
ND = DIM // 128       # 8 d-chunks
NHL = 8               # heads per core
F32 = mybir.dt.float32
BF16 = mybir.dt.bfloat16
AF = mybir.ActivationFunctionType


def _pieces(total, w=512):
    return [(o, min(w, total - o)) for o in range(0, total, w)]


def build_mla(S=2048):
    """Build the per-core Bass program (same SPMD program on all 8 cores)."""
    assert S % 1024 == 0
    SH = S // 2           # s-half width
    NT = S // 128         # number of 128-token chunks
    NP = S // 512         # number of 512-token projection pieces

    nc = bacc.Bacc()

    xT_d = nc.declare_dram_parameter("xT", [DIM, S], BF16, isOutput=False)
    w_kvc_d = nc.declare_dram_parameter("w_kvc", [DIM, LAT], BF16, isOutput=False)
    w_qc_d = nc.declare_dram_parameter("w_qc", [DIM, QR], BF16, isOutput=False)
    w_kvu_k_d = nc.declare_dram_parameter("w_kvu_k", [LAT, 512], BF16, isOutput=False)
    w_kvu_v_d = nc.declare_dram_parameter("w_kvu_v", [LAT, 512], BF16, isOutput=False)
    w_qu_d = nc.declare_dram_parameter("w_qu", [QR, 512], BF16, isOutput=False)
    w_o_d = nc.declare_dram_parameter("w_o", [512, DIM], BF16, isOutput=False)
    b_kvc_d = nc.declare_dram_parameter("b_kvc", [LAT, 1], F32, isOutput=False)
    b_qc_d = nc.declare_dram_parameter("b_qc", [128, 2], F32, isOutput=False)
    b_qu_d = nc.declare_dram_parameter("b_qu", [128, 4], F32, isOutput=False)
    b_kvu_k_d = nc.declare_dram_parameter("b_kvu_k", [128, 4], F32, isOutput=False)
    b_kvu_v_d = nc.declare_dram_parameter("b_kvu_v", [128, 512], F32, isOutput=False)
    b_o_d = nc.declare_dram_parameter("b_o", [128, DIM], F32, isOutput=False)
    out_d = nc.declare_dram_parameter("out", [S, DIM], F32, isOutput=True)

    with tile.TileContext(nc) as tc:
        with (
            tc.tile_pool(name="const", bufs=1) as const,
            tc.tile_pool(name="wts", bufs=1) as wts,
            tc.tile_pool(name="big", bufs=1) as big,
            tc.tile_pool(name="xin", bufs=4) as xin,
            tc.tile_pool(name="kvq", bufs=2) as kvq,
            tc.tile_pool(name="exd", bufs=4) as exd,
            tc.tile_pool(name="nrm", bufs=2) as nrm,
            tc.tile_pool(name="outs", bufs=3) as outs,
            tc.tile_pool(name="mps", bufs=2, space="PSUM") as mps,
        ):
            # ---- weight tiles (DMA'd inside W1, after the x DMAs) ----------
            w_kvc_sb = wts.tile([128, DIM], BF16, name="w_kvc_sb")
            w_qc_sb = wts.tile([128, ND * QR], BF16, name="w_qc_sb")
            w_kvu_k_sb = wts.tile([128, 512], BF16, name="w_kvu_k_sb")
            w_kvu_v_sb = wts.tile([128, 512], BF16, name="w_kvu_v_sb")
            w_qu_sb = wts.tile([128, 1024], BF16, name="w_qu_sb")
            w_o_sb = wts.tile([128, 4 * DIM], BF16, name="w_o_sb")
            b_kvc_sb = wts.tile([128, 1], F32, name="b_kvc_sb")
            b_qc_sb = wts.tile([128, 2], F32, name="b_qc_sb")
            b_qu_sb = wts.tile([128, 4], F32, name="b_qu_sb")
            b_kvu_k_sb = wts.tile([128, 4], F32, name="b_kvu_k_sb")
            b_kvu_v_sb = wts.tile([128, 512], F32, name="b_kvu_v_sb")
            b_o_sb = wts.tile([128, DIM], F32, name="b_o_sb")

            def emit_wdma_early():
                # merged DMAs (one instruction each); only what the latent
                # projections need right away.
                nc.sync.dma_start(
                    out=w_kvc_sb[:].rearrange("p (d c) -> p d c", c=128),
                    in_=w_kvc_d[:, :].rearrange("(d p) c -> p d c", p=128))
                nc.sync.dma_start(out=b_kvc_sb[:], in_=b_kvc_d[:, :])
                nc.sync.dma_start(
                    out=w_qc_sb[:].rearrange("p (d c) -> p d c", c=QR),
                    in_=w_qc_d[:, :].rearrange("(d p) c -> p d c", p=128))
                nc.sync.dma_start(out=b_qc_sb[:], in_=b_qc_d[:, :])

            def emit_wdma_mid():
                nc.sync.dma_start(out=w_kvu_k_sb[:], in_=w_kvu_k_d[:, :])
                nc.sync.dma_start(
                    out=w_qu_sb[:].rearrange("p (d c) -> p d c", c=512),
                    in_=w_qu_d[:, :].rearrange("(d p) c -> p d c", p=128))
                nc.sync.dma_start(out=b_qu_sb[:], in_=b_qu_d[:, :])
                nc.sync.dma_start(out=b_kvu_k_sb[:], in_=b_kvu_k_d[:, :])
                nc.sync.dma_start(out=w_kvu_v_sb[:], in_=w_kvu_v_d[:, :])
                nc.sync.dma_start(out=b_kvu_v_sb[:], in_=b_kvu_v_d[:, :])

            def emit_wdma_late():
                nc.sync.dma_start(
                    out=w_o_sb[:].rearrange("p (d c) -> p d c", c=DIM),
                    in_=w_o_d[:, :].rearrange("(d p) c -> p d c", p=128))
                nc.sync.dma_start(out=b_o_sb[:], in_=b_o_d[:, :])

            # ---- persistent products ---------------------------------------
            KT = big.tile([128, 4 * S], BF16, name="KT")
            QT = big.tile([128, 4 * S], BF16, name="QT")
            V = big.tile([128, NT * 520], BF16, name="V")
            ctxT = big.tile([128, 4 * S], BF16, name="ctxT")
            v_view = V[:].rearrange("p (k h c) -> p k h c", h=NHL, c=65)
            # ones columns of V (col 64 of each 65-wide head block)
            ones_f = const.tile([128, NT * NHL], F32, name="ones_f")
            nc.gpsimd.memset(ones_f[:], 1.0)
            nc.vector.tensor_copy(
                v_view[:, :, :, 64:65],
                ones_f[:].rearrange("p (k h o) -> p k h o", h=NHL, o=1))
            # warm the Exp activation table while the DMAs run so the first
            # real exp doesn't pay the table load
            warm = const.tile([1, 1], F32, name="warm")
            nc.scalar.activation(warm[:], ones_f[0:1, 0:1], AF.Exp)

            # ---- projection emitters ---------------------------------------
            xts = [None] * NP
            kvs = [None] * NP
            q0s = [None] * NP
            q1s = [None] * NP

            def emit_xdma(p):
                xt = xin.tile([128, ND * 512], BF16, tag="xt", name=f"xt{p}")
                xts[p] = xt
                nc.sync.dma_start(
                    out=xt[:].rearrange("p (d s) -> p d s", s=512),
                    in_=xT_d[:, 512 * p:512 * p + 512].rearrange(
                        "(d p) s -> p d s", p=128))

            def emit_lat_kv(p, pool):
                ps = pool.tile([128, 512], F32, tag="mm")
                for dc in range(ND):
                    nc.tensor.matmul(
                        ps[:], w_kvc_sb[:, 128 * dc:128 * dc + 128],
                        xts[p][:, 512 * dc:512 * dc + 512],
                        start=dc == 0, stop=dc == ND - 1)
                t = kvq.tile([128, 512], BF16, tag="kvs")
                kvs[p] = t
                nc.vector.tensor_scalar_add(t[:], ps[:], b_kvc_sb[:, 0:1])

            def emit_lat_q(p, half, pool):
                ps = pool.tile([128, 512], F32, tag="mm")
                for dc in range(ND):
                    nc.tensor.matmul(
                        ps[:], w_qc_sb[:, QR * dc + 128 * half:QR * dc + 128 * half + 128],
                        xts[p][:, 512 * dc:512 * dc + 512],
                        start=dc == 0, stop=dc == ND - 1)
                t = kvq.tile([128, 512], BF16, tag=f"q{half}s")
                (q0s if half == 0 else q1s)[p] = t
                nc.vector.tensor_scalar_add(t[:], ps[:], b_qc_sb[:, half:half + 1])

            def emit_qt(p, c, pool):
                ps = pool.tile([128, 512], F32, tag="mm")
                nc.tensor.matmul(ps[:], w_qu_sb[:, 128 * c:128 * c + 128],
                                 q0s[p][:], start=True, stop=False)
                nc.tensor.matmul(ps[:], w_qu_sb[:, 512 + 128 * c:512 + 128 * c + 128],
                                 q1s[p][:], start=False, stop=True)
                nc.vector.tensor_scalar_add(
                    QT[:, c * S + 512 * p:c * S + 512 * p + 512], ps[:],
                    b_qu_sb[:, c:c + 1])

            def emit_kt(p, c, pool):
                ps = pool.tile([128, 512], F32, tag="mm")
                nc.tensor.matmul(ps[:], w_kvu_k_sb[:, 128 * c:128 * c + 128],
                                 kvs[p][:], start=True, stop=True)
                nc.vector.tensor_scalar_add(
                    KT[:, c * S + 512 * p:c * S + 512 * p + 512], ps[:],
                    b_kvu_k_sb[:, c:c + 1])

            def emit_v(p, q, pool):
                k = 4 * p + q
                ps = pool.tile([128, 512], F32, tag="mm")
                nc.tensor.matmul(ps[:], kvs[p][:, 128 * q:128 * q + 128],
                                 w_kvu_v_sb[:], start=True, stop=True)
                nc.vector.tensor_add(
                    v_view[:, k, :, 0:64],
                    ps[:].rearrange("p (h c) -> p h c", c=64),
                    b_kvu_v_sb[:].rearrange("p (h c) -> p h c", c=64))

            def emit_outproj_half(si, o2, pool):
                ps = pool.tile([128, 512], F32, tag="mm")
                for cc in range(4):
                    nc.tensor.matmul(
                        ps[:],
                        ctxT[:, cc * S + 128 * si:cc * S + 128 * si + 128],
                        w_o_sb[:, DIM * cc + o2:DIM * cc + o2 + 512],
                        start=cc == 0, stop=cc == 3)
                ob = obs[si % 4]
                nc.vector.tensor_add(ob[:, o2:o2 + 512], ps[:],
                                     b_o_sb[:, o2:o2 + 512])
                if o2 == 512:
                    nc.sync.dma_start(
                        out=out_d[128 * si:128 * si + 128, :], in_=ob[:])

            obs = [None] * 4

            def outproj_steps(si_range):
                for si in si_range:
                    obs[si % 4] = outs.tile([128, DIM], F32, tag="ob",
                                            name=f"ob{si}")
                    yield lambda pool, si=si: emit_outproj_half(si, 0, pool)
                    yield lambda pool, si=si: emit_outproj_half(si, 512, pool)

            # ---- attention ------------------------------------------------
            def attn_head(j, h, scp, ctp, fill, need=None):
                """Attention for s-half j, local head h.  PV is emitted one
                k-iteration late so the PE never puts PV(k)/QK(k+1) between
                exp(k) and exp(k+1) on the critical path — the scalar engine
                (exp) is the attention-phase bottleneck and must never wait.
                fill() interleaves projection/out_proj PE work; need(k) drains
                fills that iteration k depends on."""
                s0 = SH * j
                c = h // 2
                po = 64 * (h % 2)
                kmax = (SH // 128) * (j + 1)
                nbank = SH // 512
                last_k = {bi: min(kmax - 1, (s0 + 512 * (bi + 1)) // 128 - 1)
                          for bi in range(nbank)}
                ctx = ctp.tile([65, SH], F32, tag="ctx")

                def emit_pv(k, rel, ex):
                    for bi in range(nbank):
                        a = max(rel, 512 * bi)
                        b2 = min(SH, 512 * bi + 512)
                        if a >= b2:
                            continue
                        nc.tensor.matmul(
                            ctx[:, a:b2],
                            V[:, 520 * k + 65 * h:520 * k + 65 * h + 65],
                            ex[:, a - rel:b2 - rel],
                            start=(k == 0), stop=(k == last_k[bi]))

                pend = None
                for k in range(kmax):
                    if need is not None:
                        need(k)
                    t0 = 128 * k
                    ss = max(s0, t0)
                    fd = s0 + SH - ss
                    rel = ss - s0
                    sc = scp.tile([128, SH], F32, tag="sc")
                    for o2, w2 in _pieces(fd):
                        nc.tensor.matmul(
                            sc[:, o2:o2 + w2],
                            KT[po:po + 64, c * S + t0:c * S + t0 + 128],
                            QT[po:po + 64, c * S + ss + o2:c * S + ss + o2 + w2],
                            start=True, stop=True)
                    ex = exd.tile([128, SH], BF16, tag="ex")
                    nc.scalar.activation(ex[:, :fd], sc[:, :fd], AF.Exp,
                                         scale=0.125)
                    if t0 >= s0:
                        nc.gpsimd.affine_select(
                            out=ex[:, 0:128], in_=ex[:, 0:128],
                            pattern=[[1, 128]],
                            compare_op=mybir.AluOpType.is_ge,
                            fill=0.0, base=0, channel_multiplier=-1)
                    if pend is not None:
                        emit_pv(*pend)
                    pend = (k, rel, ex)
                    fill()
                emit_pv(*pend)
                # normalize: ctx[0:64] * (1/ctx[64]) -> ctxT
                rec = nrm.tile([1, SH], F32, tag="rec")
                nc.vector.reciprocal(rec[:], ctx[64:65, :])
                rbc = nrm.tile([64, SH], F32, tag="rbc")
                nc.gpsimd.partition_broadcast(rbc[:], rec[0:1, :])
                nc.vector.tensor_mul(
                    ctxT[po:po + 64, c * S + s0:c * S + s0 + SH],
                    ctx[0:64, :], rbc[:])

            # ================= W1: minimal projection prefix =================
            # Just enough that head 0's j=0 attention can start: q-latents for
            # pieces 0,1, the c=0 K/Q up-projections, V chunk 0.  Everything
            # else becomes interleaved fill work.
            with tc.tile_pool(name="w1p", bufs=4, space="PSUM") as w1p:
                emit_xdma(0)
                emit_wdma_early()
                emit_xdma(1)
                emit_wdma_mid()
                emit_xdma(2)
                emit_xdma(3)
                emit_wdma_late()
                emit_lat_kv(0, w1p)
                emit_lat_q(0, 0, w1p)
                emit_lat_q(0, 1, w1p)
                emit_lat_q(1, 0, w1p)
                emit_lat_q(1, 1, w1p)
                emit_kt(0, 0, w1p)
                emit_qt(0, 0, w1p)
                emit_qt(1, 0, w1p)
                emit_v(0, 0, w1p)

            # fill steps for the j=0 attention window, in dependency order;
            # labels mark the last step each attention point requires.
            w2_fills = []

            def F(label, fn):
                w2_fills.append((label, fn))

            F("h0i1", lambda pool: emit_lat_kv(1, pool))
            F("h0i2", lambda pool: emit_v(0, 1, pool))
            F("h0i3", lambda pool: emit_v(0, 2, pool))
            F("h0i4", lambda pool: emit_kt(1, 0, pool))
            F("h0i4", lambda pool: emit_v(0, 3, pool))
            F("h0i5", lambda pool: emit_v(1, 0, pool))
            F("h0i6", lambda pool: emit_v(1, 1, pool))
            F("h0i7", lambda pool: emit_v(1, 2, pool))
            F("h0i7", lambda pool: emit_v(1, 3, pool))
            for c in (1, 2, 3):
                for p in (0, 1):
                    F(f"kq{c}", lambda pool, p=p, c=c: emit_qt(p, c, pool))
                    F(f"kq{c}", lambda pool, p=p, c=c: emit_kt(p, c, pool))
            for p in (2, 3):
                F("p23", lambda pool, p=p: emit_lat_kv(p, pool))
                F("p23", lambda pool, p=p: emit_lat_q(p, 0, pool))
                F("p23", lambda pool, p=p: emit_lat_q(p, 1, pool))
                for c in range(4):
                    F("p23", lambda pool, p=p, c=c: emit_qt(p, c, pool))
                    F("p23", lambda pool, p=p, c=c: emit_kt(p, c, pool))
                for q in range(4):
                    F("p23", lambda pool, p=p, q=q: emit_v(p, q, pool))

            fill_pos = [0]

            def drain(n):
                while n > 0 and fill_pos[0] < len(w2_fills):
                    w2_fills[fill_pos[0]][1](mps)
                    fill_pos[0] += 1
                    n -= 1

            def drain_until(label):
                idx = max((i for i, (lb, _) in enumerate(w2_fills) if lb == label),
                          default=-1)
                while fill_pos[0] <= idx:
                    w2_fills[fill_pos[0]][1](mps)
                    fill_pos[0] += 1

            with (
                tc.tile_pool(name="scp", bufs=2, space="PSUM") as scp,
                tc.tile_pool(name="ctp", bufs=1, space="PSUM") as ctp,
            ):
                # ================= W2: attention j=0 + fills ================
                def need_h0(k):
                    drain_until(f"h0i{k}")

                for h in range(NHL):
                    if h >= 2:
                        drain_until(f"kq{h // 2}")
                    if h == 1:
                        drain_until("h0i7")
                    attn_head(0, h, scp, ctp, lambda: drain(1),
                              need=need_h0 if h == 0 else None)
                drain(len(w2_fills))

                # ================= W3: attention j=1 + out_proj j0 ==========
                w3 = outproj_steps(range(8))
                w3_done = [False]

                def drain3():
                    if not w3_done[0]:
                        try:
                            next(w3)(mps)
                        except StopIteration:
                            w3_done[0] = True

                for h in range(NHL):
                    attn_head(1, h, scp, ctp, drain3)
                while not w3_done[0]:
                    drain3()

            # ================= W4: out_proj j1 tail =========================
            for step in outproj_steps(range(8, 16)):
                step(mps)

    nc.finalize()
    return nc


def shard_inputs(inputs, S=2048):
    """Build the 8 per-core input maps from full inputs (host-side prep:
    transpose x, cast matmul operands to bf16, pre-broadcast row biases)."""
    import ml_dtypes
    bf = lambda a: np.ascontiguousarray(np.asarray(a)).astype(ml_dtypes.bfloat16)
    f = lambda a: np.ascontiguousarray(np.asarray(a, dtype=np.float32))
    x = np.asarray(inputs["x"], dtype=np.float32)
    w_kvc, b_kvc = inputs["w_kvc"], f(inputs["b_kvc"])
    w_kvu, b_kvu = np.asarray(inputs["w_kvu"]), f(inputs["b_kvu"])
    w_qc, b_qc = inputs["w_qc"], f(inputs["b_qc"])
    w_qu, b_qu = np.asarray(inputs["w_qu"]), f(inputs["b_qu"])
    w_o, b_o = np.asarray(inputs["w_o"]), f(inputs["b_o"])
    xT = [bf(x[b].T) for b in range(B)]
    w_kvc_b = bf(w_kvc)
    w_qc_b = bf(w_qc)
    in_maps = []
    for core in range(NCORES):
        b = core // 2
        g = core % 2
        cs = slice(512 * g, 512 * g + 512)
        in_maps.append({
            "xT": xT[b],
            "w_kvc": w_kvc_b,
            "w_qc": w_qc_b,
            "w_kvu_k": bf(w_kvu[:, cs]),
            "w_kvu_v": bf(w_kvu[:, 1024 + 512 * g:1024 + 512 * g + 512]),
            "w_qu": bf(w_qu[:, cs]),
            "w_o": bf(w_o[cs, :]),
            "b_kvc": b_kvc.reshape(LAT, 1),
            "b_qc": np.ascontiguousarray(b_qc.reshape(2, 128).T),
            "b_qu": np.ascontiguousarray(b_qu[cs].reshape(4, 128).T),
            "b_kvu_k": np.ascontiguousarray(b_kvu[cs].reshape(4, 128).T),
            "b_kvu_v": np.ascontiguousarray(np.broadcast_to(
                b_kvu[1024 + 512 * g:1024 + 512 * g + 512], (128, 512))),
            "b_o": np.ascontiguousarray(np.broadcast_to(b_o * 0.5, (128, DIM))),
        })
    return in_maps


def kernel(**inputs) -> np.ndarray:
    from concourse.bass_utils import run_bass_kernel_spmd

    x = np.asarray(inputs["x"])
    S = x.shape[1]
    nc = build_mla(S=S)
    in_maps = shard_inputs(inputs, S=S)
    res = run_bass_kernel_spmd(nc, in_maps, list(range(NCORES))).results
    out = np.empty((B, S, DIM), dtype=np.float32)
    for b in range(B):
        out[b] = res[2 * b]["out"] + res[2 * b + 1]["out"]
    return out


# revision 16
# speedup vs baseline: 1.2877x; 1.0123x over previous
"""MLA (multi-head latent attention) Bass kernel for Trainium2, 8 NeuronCores.

Sharding: core i handles batch b = i // 2 and head-group g = i % 2
(8 of the 16 heads).  Each core computes a partial output
(its heads' contribution through out_proj, plus b_o/2); the host sums
the two partials per batch.

v2 design notes (all driven by the CoreSim v1 cost model):
  - The host pre-transposes x to [dim, S] and pre-casts x + all weights
    to bf16, so there are no on-chip PE transposes and no weight staging
    copies.  bf16 matmuls are 1 cycle/row at any moving width (f32r
    degrades 4x below 256), and halve SBUF/DMA traffic.
  - Row-vector biases (b_kvu_v, b_o) arrive pre-broadcast as [128, n]
    tiles and are folded into the PSUM->SBUF drain via tensor_add on the
    DVE, so the PE never runs bias matmuls.  Per-partition biases
    (b_kvc/b_qc/b_qu/b_kvu_k) fold into tensor_scalar_add drains.
  - Layout: everything "t-major" (feature dim on partitions):
      xT [8x128, S] (DMA'd), kv_latT [128,S], q_latT [256,S],
      KT/QT [128 = 2 heads x 64, 4 chunks x S] bf16,
      V [128 tokens, 16 chunks x (8 heads x 65)] bf16 with a ones
      column per head (col 64) that accumulates the softmax denominator
      during the PV matmul (row 64 of ctx PSUM).
  - Attention per (s-half j, head h): stream key chunks k; QK scores to
    PSUM, exp(s/8) on the scalar engine (the only Activation-engine
    work in the kernel), causal diagonal via gpsimd affine_select, PV
    accumulates ctx^T[65, SH] in PSUM; normalize via DVE reciprocal +
    gpsimd partition_broadcast + DVE multiply into ctxT bf16.
  - Software pipelining by emission order: a minimal projection prefix,
    then attention j=0 interleaved with the remaining projection work,
    then attention j=1 interleaved with out_proj of the first half,
    then the out_proj tail.  Interleaved "fill" work keeps the PE busy
    while the scalar engine (the attention-phase bottleneck) runs exp.
  - PSUM budget (8 banks): sc slots 2x[128,1024]f32 = 4, ctx
    [65,1024]f32 = 2, shared misc pool [128,512]f32 x2 = 2.
"""

import numpy as np

import concourse.bass as bass
import concourse.bacc as bacc
import concourse.mybir as mybir
import concourse.tile as tile

DIM = 1024
NUM_HEADS = 16
HEAD_DIM = 64
LAT = 128
QR = 256
B = 4
NCORES = 8
ND = DIM // 128       # 8 d-chunks
NHL = 8               # heads per core
F32 = mybir.dt.float32
BF16 = mybir.dt.bfloat16
AF = mybir.ActivationFunctionType


def _pieces(total, w=512):
    return [(o, min(w, total - o)) for o in range(0, total, w)]


def build_mla(S=2048):
    """Build the per-core Bass program (same SPMD program on all 8 cores)."""
    assert S % 1024 == 0
    SH = S // 2           # s-half width
    NT = S // 128         # number of 128-token chunks
    NP = S // 512         # number of 512-token projection pieces

    nc = bacc.Bacc()

    xT_d = nc.declare_dram_parameter("xT", [DIM, S], BF16, isOutput=False)
    w_kvc_d = nc.declare_dram_parameter("w_kvc", [DIM, LAT], BF16, isOutput=False)
    w_qc_d = nc.declare_dram_parameter("w_qc", [DIM, QR], BF16, isOutput=False)
    w_kvu_k_d = nc.declare_dram_parameter("w_kvu_k", [LAT, 512], BF16, isOutput=False)
    w_kvu_v_d = nc.declare_dram_parameter("w_kvu_v", [LAT, 512], BF16, isOutput=False)
    w_qu_d = nc.declare_dram_parameter("w_qu", [QR, 512], BF16, isOutput=False)
    w_o_d = nc.declare_dram_parameter("w_o", [512, DIM], BF16, isOutput=False)
    b_kvc_d = nc.declare_dram_parameter("b_kvc", [LAT, 1], F32, isOutput=False)
    b_qc_d = nc.declare_dram_parameter("b_qc", [128, 2], F32, isOutput=False)
    b_qu_d = nc.declare_dram_parameter("b_qu", [128, 4], F32, isOutput=False)
    b_kvu_k_d = nc.declare_dram_parameter("b_kvu_k", [128, 4], F32, isOutput=False)
    b_kvu_v_d = nc.declare_dram_parameter("b_kvu_v", [128, 512], F32, isOutput=False)
    b_o_d = nc.declare_dram_parameter("b_o", [128, DIM], F32, isOutput=False)
    out_d = nc.declare_dram_parameter("out", [S, DIM], F32, isOutput=True)

    with tile.TileContext(nc) as tc:
        with (
            tc.tile_pool(name="const", bufs=1) as const,
            tc.tile_pool(name="wts", bufs=1) as wts,
            tc.tile_pool(name="big", bufs=1) as big,
            tc.tile_pool(name="xin", bufs=4) as xin,
            tc.tile_pool(name="kvq", bufs=2) as kvq,
            tc.tile_pool(name="exd", bufs=6) as exd,
            tc.tile_pool(name="nrm", bufs=2) as nrm,
            tc.tile_pool(name="outs", bufs=3) as outs,
            tc.tile_pool(name="mps", bufs=2, space="PSUM") as mps,
        ):
            # ---- weight tiles (DMA'd inside W1, after the x DMAs) ----------
            w_kvc_sb = wts.tile([128, DIM], BF16, name="w_kvc_sb")
            w_qc_sb = wts.tile([128, ND * QR], BF16, name="w_qc_sb")
            w_kvu_k_sb = wts.tile([128, 512], BF16, name="w_kvu_k_sb")
            w_kvu_v_sb = wts.tile([128, 512], BF16, name="w_kvu_v_sb")
            w_qu_sb = wts.tile([128, 1024], BF16, name="w_qu_sb")
            w_o_sb = wts.tile([128, 4 * DIM], BF16, name="w_o_sb")
            b_kvc_sb = wts.tile([128, 1], F32, name="b_kvc_sb")
            b_qc_sb = wts.tile([128, 2], F32, name="b_qc_sb")
            b_qu_sb = wts.tile([128, 4], F32, name="b_qu_sb")
            b_kvu_k_sb = wts.tile([128, 4], F32, name="b_kvu_k_sb")
            b_kvu_v_sb = wts.tile([128, 512], F32, name="b_kvu_v_sb")
            b_o_sb = wts.tile([128, DIM], F32, name="b_o_sb")

            def emit_wdma_early():
                # merged DMAs (one instruction each); only what the latent
                # projections need right away.
                nc.sync.dma_start(
                    out=w_kvc_sb[:].rearrange("p (d c) -> p d c", c=128),
                    in_=w_kvc_d[:, :].rearrange("(d p) c -> p d c", p=128))
                nc.sync.dma_start(out=b_kvc_sb[:], in_=b_kvc_d[:, :])
                nc.sync.dma_start(
                    out=w_qc_sb[:].rearrange("p (d c) -> p d c", c=QR),
                    in_=w_qc_d[:, :].rearrange("(d p) c -> p d c", p=128))
                nc.sync.dma_start(out=b_qc_sb[:], in_=b_qc_d[:, :])

            def emit_wdma_mid():
                nc.sync.dma_start(out=w_kvu_k_sb[:], in_=w_kvu_k_d[:, :])
                nc.sync.dma_start(
                    out=w_qu_sb[:].rearrange("p (d c) -> p d c", c=512),
                    in_=w_qu_d[:, :].rearrange("(d p) c -> p d c", p=128))
                nc.sync.dma_start(out=b_qu_sb[:], in_=b_qu_d[:, :])
                nc.sync.dma_start(out=b_kvu_k_sb[:], in_=b_kvu_k_d[:, :])
                nc.sync.dma_start(out=w_kvu_v_sb[:], in_=w_kvu_v_d[:, :])
                nc.sync.dma_start(out=b_kvu_v_sb[:], in_=b_kvu_v_d[:, :])

            def emit_wdma_late():
                nc.sync.dma_start(
                    out=w_o_sb[:].rearrange("p (d c) -> p d c", c=DIM),
                    in_=w_o_d[:, :].rearrange("(d p) c -> p d c", p=128))
                nc.sync.dma_start(out=b_o_sb[:], in_=b_o_d[:, :])

            # ---- persistent products ---------------------------------------
            KT = big.tile([128, 4 * S], BF16, name="KT")
            QT = big.tile([128, 4 * S], BF16, name="QT")
            V = big.tile([128, NT * 520], BF16, name="V")
            ctxT = big.tile([128, 4 * S], BF16, name="ctxT")
            v_view = V[:].rearrange("p (k h c) -> p k h c", h=NHL, c=65)
            # ones columns of V (col 64 of each 65-wide head block)
            ones_f = const.tile([128, NT * NHL], F32, name="ones_f")
            nc.gpsimd.memset(ones_f[:], 1.0)
            nc.vector.tensor_copy(
                v_view[:, :, :, 64:65],
                ones_f[:].rearrange("p (k h o) -> p k h o", h=NHL, o=1))
            # warm the Exp activation table while the DMAs run so the first
            # real exp doesn't pay the table load
            warm = const.tile([1, 1], F32, name="warm")
            nc.scalar.activation(warm[:], ones_f[0:1, 0:1], AF.Exp)

            # ---- projection emitters ---------------------------------------
            xts = [None] * NP
            kvs = [None] * NP
            q0s = [None] * NP
            q1s = [None] * NP

            def emit_xdma(p):
                xt = xin.tile([128, ND * 512], BF16, tag="xt", name=f"xt{p}")
                xts[p] = xt
                nc.sync.dma_start(
                    out=xt[:].rearrange("p (d s) -> p d s", s=512),
                    in_=xT_d[:, 512 * p:512 * p + 512].rearrange(
                        "(d p) s -> p d s", p=128))

            def emit_lat_kv(p, pool):
                ps = pool.tile([128, 512], F32, tag="mm")
                for dc in range(ND):
                    nc.tensor.matmul(
                        ps[:], w_kvc_sb[:, 128 * dc:128 * dc + 128],
                        xts[p][:, 512 * dc:512 * dc + 512],
                        start=dc == 0, stop=dc == ND - 1)
                t = kvq.tile([128, 512], BF16, tag="kvs")
                kvs[p] = t
                nc.vector.tensor_scalar_add(t[:], ps[:], b_kvc_sb[:, 0:1])

            def emit_lat_q(p, half, pool):
                ps = pool.tile([128, 512], F32, tag="mm")
                for dc in range(ND):
                    nc.tensor.matmul(
                        ps[:], w_qc_sb[:, QR * dc + 128 * half:QR * dc + 128 * half + 128],
                        xts[p][:, 512 * dc:512 * dc + 512],
                        start=dc == 0, stop=dc == ND - 1)
                t = kvq.tile([128, 512], BF16, tag=f"q{half}s")
                (q0s if half == 0 else q1s)[p] = t
                nc.vector.tensor_scalar_add(t[:], ps[:], b_qc_sb[:, half:half + 1])

            def emit_qt(p, c, pool):
                ps = pool.tile([128, 512], F32, tag="mm")
                nc.tensor.matmul(ps[:], w_qu_sb[:, 128 * c:128 * c + 128],
                                 q0s[p][:], start=True, stop=False)
                nc.tensor.matmul(ps[:], w_qu_sb[:, 512 + 128 * c:512 + 128 * c + 128],
                                 q1s[p][:], start=False, stop=True)
                nc.vector.tensor_scalar_add(
                    QT[:, c * S + 512 * p:c * S + 512 * p + 512], ps[:],
                    b_qu_sb[:, c:c + 1])

            def emit_kt(p, c, pool):
                ps = pool.tile([128, 512], F32, tag="mm")
                nc.tensor.matmul(ps[:], w_kvu_k_sb[:, 128 * c:128 * c + 128],
                                 kvs[p][:], start=True, stop=True)
                nc.vector.tensor_scalar_add(
                    KT[:, c * S + 512 * p:c * S + 512 * p + 512], ps[:],
                    b_kvu_k_sb[:, c:c + 1])

            def emit_v(p, q, pool):
                k = 4 * p + q
                ps = pool.tile([128, 512], F32, tag="mm")
                nc.tensor.matmul(ps[:], kvs[p][:, 128 * q:128 * q + 128],
                                 w_kvu_v_sb[:], start=True, stop=True)
                nc.vector.tensor_add(
                    v_view[:, k, :, 0:64],
                    ps[:].rearrange("p (h c) -> p h c", c=64),
                    b_kvu_v_sb[:].rearrange("p (h c) -> p h c", c=64))

            def emit_outproj_half(si, o2, pool):
                ps = pool.tile([128, 512], F32, tag="mm")
                for cc in range(4):
                    nc.tensor.matmul(
                        ps[:],
                        ctxT[:, cc * S + 128 * si:cc * S + 128 * si + 128],
                        w_o_sb[:, DIM * cc + o2:DIM * cc + o2 + 512],
                        start=cc == 0, stop=cc == 3)
                ob = obs[si % 4]
                nc.vector.tensor_add(ob[:, o2:o2 + 512], ps[:],
                                     b_o_sb[:, o2:o2 + 512])
                if o2 == 512:
                    nc.sync.dma_start(
                        out=out_d[128 * si:128 * si + 128, :], in_=ob[:])

            obs = [None] * 4

            def outproj_steps(si_range):
                for si in si_range:
                    obs[si % 4] = outs.tile([128, DIM], F32, tag="ob",
                                            name=f"ob{si}")
                    yield lambda pool, si=si: emit_outproj_half(si, 0, pool)
                    yield lambda pool, si=si: emit_outproj_half(si, 512, pool)

            # ---- attention ------------------------------------------------
            def attn_head(j, h, scp, ctp, fill, need=None):
                """Attention for s-half j, local head h.  PV is emitted one
                k-iteration late so the PE never puts PV(k)/QK(k+1) between
                exp(k) and exp(k+1) on the critical path — the scalar engine
                (exp) is the attention-phase bottleneck and must never wait.
                fill() interleaves projection/out_proj PE work; need(k) drains
                fills that iteration k depends on."""
                s0 = SH * j
                c = h // 2
                po = 64 * (h % 2)
                kmax = (SH // 128) * (j + 1)
                nbank = SH // 512
                last_k = {bi: min(kmax - 1, (s0 + 512 * (bi + 1)) // 128 - 1)
                          for bi in range(nbank)}
                ctx = ctp.tile([65, SH], F32, tag="ctx")

                def emit_pv(k, rel, ex):
                    for bi in range(nbank):
                        a = max(rel, 512 * bi)
                        b2 = min(SH, 512 * bi + 512)
                        if a >= b2:
                            continue
                        nc.tensor.matmul(
                            ctx[:, a:b2],
                            V[:, 520 * k + 65 * h:520 * k + 65 * h + 65],
                            ex[:, a - rel:b2 - rel],
                            start=(k == 0), stop=(k == last_k[bi]))

                pend = []
                for k in range(kmax):
                    if need is not None:
                        need(k)
                    t0 = 128 * k
                    ss = max(s0, t0)
                    fd = s0 + SH - ss
                    rel = ss - s0
                    sc = scp.tile([128, SH], F32, tag="sc")
                    for o2, w2 in _pieces(fd):
                        nc.tensor.matmul(
                            sc[:, o2:o2 + w2],
                            KT[po:po + 64, c * S + t0:c * S + t0 + 128],
                            QT[po:po + 64, c * S + ss + o2:c * S + ss + o2 + w2],
                            start=True, stop=True)
                    ex = exd.tile([128, SH], BF16, tag="ex")
                    nc.scalar.activation(ex[:, :fd], sc[:, :fd], AF.Exp,
                                         scale=0.125)
                    if t0 >= s0:
                        nc.gpsimd.affine_select(
                            out=ex[:, 0:128], in_=ex[:, 0:128],
                            pattern=[[1, 128]],
                            compare_op=mybir.AluOpType.is_ge,
                            fill=0.0, base=0, channel_multiplier=-1)
                    if len(pend) == 2:
                        emit_pv(*pend.pop(0))
                    pend.append((k, rel, ex))
                    fill()
                for a in pend:
                    emit_pv(*a)
                # normalize: ctx[0:64] * (1/ctx[64]) -> ctxT, in two
                # column-halves so the ctx PSUM slot releases sooner
                for o2 in (0, SH // 2):
                    hs = SH // 2
                    rec = nrm.tile([1, SH // 2], F32, tag="rec")
                    nc.vector.reciprocal(rec[:], ctx[64:65, o2:o2 + hs])
                    rbc = nrm.tile([64, SH // 2], F32, tag="rbc")
                    nc.gpsimd.partition_broadcast(rbc[:], rec[0:1, :])
                    nc.vector.tensor_mul(
                        ctxT[po:po + 64, c * S + s0 + o2:c * S + s0 + o2 + hs],
                        ctx[0:64, o2:o2 + hs], rbc[:])

            # ================= W1: minimal projection prefix =================
            # Just enough that head 0's j=0 attention can start: q-latents for
            # pieces 0,1, the c=0 K/Q up-projections, V chunk 0.  Everything
            # else becomes interleaved fill work.
            with tc.tile_pool(name="w1p", bufs=4, space="PSUM") as w1p:
                emit_xdma(0)
                emit_wdma_early()
                emit_xdma(1)
                emit_wdma_mid()
                emit_xdma(2)
                emit_xdma(3)
                emit_wdma_late()
                emit_lat_kv(0, w1p)
                emit_lat_q(0, 0, w1p)
                emit_lat_q(0, 1, w1p)
                emit_lat_q(1, 0, w1p)
                emit_lat_q(1, 1, w1p)
                emit_kt(0, 0, w1p)
                emit_qt(0, 0, w1p)
                emit_qt(1, 0, w1p)
                emit_v(0, 0, w1p)

            # fill steps for the j=0 attention window, in dependency order;
            # labels mark the last step each attention point requires.
            w2_fills = []

            def F(label, fn):
                w2_fills.append((label, fn))

            F("h0i1", lambda pool: emit_lat_kv(1, pool))
            F("h0i2", lambda pool: emit_v(0, 1, pool))
            F("h0i3", lambda pool: emit_v(0, 2, pool))
            F("h0i4", lambda pool: emit_kt(1, 0, pool))
            F("h0i4", lambda pool: emit_v(0, 3, pool))
            F("h0i5", lambda pool: emit_v(1, 0, pool))
            F("h0i6", lambda pool: emit_v(1, 1, pool))
            F("h0i7", lambda pool: emit_v(1, 2, pool))
            F("h0i7", lambda pool: emit_v(1, 3, pool))
            for c in (1, 2, 3):
                for p in (0, 1):
                    F(f"kq{c}", lambda pool, p=p, c=c: emit_qt(p, c, pool))
                    F(f"kq{c}", lambda pool, p=p, c=c: emit_kt(p, c, pool))
            for p in (2, 3):
                F("p23", lambda pool, p=p: emit_lat_kv(p, pool))
                F("p23", lambda pool, p=p: emit_lat_q(p, 0, pool))
                F("p23", lambda pool, p=p: emit_lat_q(p, 1, pool))
                for c in range(4):
                    F("p23", lambda pool, p=p, c=c: emit_qt(p, c, pool))
                    F("p23", lambda pool, p=p, c=c: emit_kt(p, c, pool))
                for q in range(4):
                    F("p23", lambda pool, p=p, q=q: emit_v(p, q, pool))

            fill_pos = [0]

            def drain(n):
                while n > 0 and fill_pos[0] < len(w2_fills):
                    w2_fills[fill_pos[0]][1](mps)
                    fill_pos[0] += 1
                    n -= 1

            def drain_until(label):
                idx = max((i for i, (lb, _) in enumerate(w2_fills) if lb == label),
                          default=-1)
                while fill_pos[0] <= idx:
                    w2_fills[fill_pos[0]][1](mps)
                    fill_pos[0] += 1

            with (
                tc.tile_pool(name="scp", bufs=2, space="PSUM") as scp,
                tc.tile_pool(name="ctp", bufs=1, space="PSUM") as ctp,
            ):
                # ================= W2: attention j=0 + fills ================
                def need_h0(k):
                    drain_until(f"h0i{k}")

                for h in range(NHL):
                    if h >= 2:
                        drain_until(f"kq{h // 2}")
                    if h == 1:
                        drain_until("h0i7")
                    attn_head(0, h, scp, ctp, lambda: drain(1),
                              need=need_h0 if h == 0 else None)
                drain(len(w2_fills))

                # ================= W3: attention j=1 + out_proj j0 ==========
                w3 = outproj_steps(range(8))
                w3_done = [False]

                def drain3():
                    if not w3_done[0]:
                        try:
                            next(w3)(mps)
                        except StopIteration:
                            w3_done[0] = True

                for h in range(NHL):
                    attn_head(1, h, scp, ctp, drain3)
                while not w3_done[0]:
                    drain3()

            # ================= W4: out_proj j1 tail =========================
            for step in outproj_steps(range(8, 16)):
                step(mps)

    nc.finalize()
    return nc


def shard_inputs(inputs, S=2048):
    """Build the 8 per-core input maps from full inputs (host-side prep:
    transpose x, cast matmul operands to bf16, pre-broadcast row biases)."""
    import ml_dtypes
    bf = lambda a: np.ascontiguousarray(np.asarray(a)).astype(ml_dtypes.bfloat16)
    f = lambda a: np.ascontiguousarray(np.asarray(a, dtype=np.float32))
    x = np.asarray(inputs["x"], dtype=np.float32)
    w_kvc, b_kvc = inputs["w_kvc"], f(inputs["b_kvc"])
    w_kvu, b_kvu = np.asarray(inputs["w_kvu"]), f(inputs["b_kvu"])
    w_qc, b_qc = inputs["w_qc"], f(inputs["b_qc"])
    w_qu, b_qu = np.asarray(inputs["w_qu"]), f(inputs["b_qu"])
    w_o, b_o = np.asarray(inputs["w_o"]), f(inputs["b_o"])
    xT = [bf(x[b].T) for b in range(B)]
    w_kvc_b = bf(w_kvc)
    w_qc_b = bf(w_qc)
    in_maps = []
    for core in range(NCORES):
        b = core // 2
        g = core % 2
        cs = slice(512 * g, 512 * g + 512)
        in_maps.append({
            "xT": xT[b],
            "w_kvc": w_kvc_b,
            "w_qc": w_qc_b,
            "w_kvu_k": bf(w_kvu[:, cs]),
            "w_kvu_v": bf(w_kvu[:, 1024 + 512 * g:1024 + 512 * g + 512]),
            "w_qu": bf(w_qu[:, cs]),
            "w_o": bf(w_o[cs, :]),
            "b_kvc": b_kvc.reshape(LAT, 1),
            "b_qc": np.ascontiguousarray(b_qc.reshape(2, 128).T),
            "b_qu": np.ascontiguousarray(b_qu[cs].reshape(4, 128).T),
            "b_kvu_k": np.ascontiguousarray(b_kvu[cs].reshape(4, 128).T),
            "b_kvu_v": np.ascontiguousarray(np.broadcast_to(
                b_kvu[1024 + 512 * g:1024 + 512 * g + 512], (128, 512))),
            "b_o": np.ascontiguousarray(np.broadcast_to(b_o * 0.5, (128, DIM))),
        })
    return in_maps


def kernel(**inputs) -> np.ndarray:
    from concourse.bass_utils import run_bass_kernel_spmd

    x = np.asarray(inputs["x"])
    S = x.shape[1]
    nc = build_mla(S=S)
    in_maps = shard_inputs(inputs, S=S)
    res = run_bass_kernel_spmd(nc, in_maps, list(range(NCORES))).results
    out = np.empty((B, S, DIM), dtype=np.float32)
    for b in range(B):
        out[b] = res[2 * b]["out"] + res[2 * b + 1]["out"]
    return out


# revision 20
# speedup vs baseline: 1.3505x; 1.0488x over previous
"""MLA (multi-head latent attention) Bass kernel for Trainium2, 8 NeuronCores.

Sharding: core i handles batch b = i // 2 and head-group g = i % 2
(8 of the 16 heads).  Each core computes a partial output
(its heads' contribution through out_proj, plus b_o/2); the host sums
the two partials per batch.

v2 design notes (all driven by the CoreSim v1 cost model):
  - The host pre-transposes x to [dim, S] and pre-casts x + all weights
    to bf16, so there are no on-chip PE transposes and no weight staging
    copies.  bf16 matmuls are 1 cycle/row at any moving width (f32r
    degrades 4x below 256), and halve SBUF/DMA traffic.
  - Row-vector biases (b_kvu_v, b_o) arrive pre-broadcast as [128, n]
    tiles and are folded into the PSUM->SBUF drain via tensor_add on the
    DVE, so the PE never runs bias matmuls.  Per-partition biases
    (b_kvc/b_qc/b_qu/b_kvu_k) fold into tensor_scalar_add drains.
  - Layout: everything "t-major" (feature dim on partitions):
      xT [8x128, S] (DMA'd), kv_latT [128,S], q_latT [256,S],
      KT/QT [128 = 2 heads x 64, 4 chunks x S] bf16,
      V [128 tokens, 16 chunks x (8 heads x 65)] bf16 with a ones
      column per head (col 64) that accumulates the softmax denominator
      during the PV matmul (row 64 of ctx PSUM).
  - Attention per (s-half j, head h): stream key chunks k; QK scores to
    PSUM, exp(s/8) on the scalar engine (the only Activation-engine
    work in the kernel), causal diagonal via gpsimd affine_select, PV
    accumulates ctx^T[65, SH] in PSUM; normalize via DVE reciprocal +
    gpsimd partition_broadcast + DVE multiply into ctxT bf16.
  - Software pipelining by emission order: a minimal projection prefix,
    then attention j=0 interleaved with the remaining projection work,
    then attention j=1 interleaved with out_proj of the first half,
    then the out_proj tail.  Interleaved "fill" work keeps the PE busy
    while the scalar engine (the attention-phase bottleneck) runs exp.
  - PSUM budget (8 banks): sc slots 2x[128,1024]f32 = 4, ctx
    [65,1024]f32 = 2, shared misc pool [128,512]f32 x2 = 2.
"""

import numpy as np

import concourse.bass as bass
import concourse.bacc as bacc
import concourse.mybir as mybir
import concourse.tile as tile

DIM = 1024
NUM_HEADS = 16
HEAD_DIM = 64
LAT = 128
QR = 256
B = 4
NCORES = 8
ND = DIM // 128       # 8 d-chunks
NHL = 8               # heads per core
F32 = mybir.dt.float32
BF16 = mybir.dt.bfloat16
AF = mybir.ActivationFunctionType


def _pieces(total, w=512):
    return [(o, min(w, total - o)) for o in range(0, total, w)]


def build_mla(S=2048):
    """Build the per-core Bass program (same SPMD program on all 8 cores)."""
    assert S % 1024 == 0
    SH = S // 2           # s-half width
    NT = S // 128         # number of 128-token chunks
    NP = S // 512         # number of 512-token projection pieces

    nc = bacc.Bacc()

    xT_d = nc.declare_dram_parameter("xT", [DIM, S], BF16, isOutput=False)
    w_kvc_d = nc.declare_dram_parameter("w_kvc", [DIM, LAT], BF16, isOutput=False)
    w_qc_d = nc.declare_dram_parameter("w_qc", [DIM, QR], BF16, isOutput=False)
    w_kvu_k_d = nc.declare_dram_parameter("w_kvu_k", [LAT, 512], BF16, isOutput=False)
    w_kvu_v_d = nc.declare_dram_parameter("w_kvu_v", [LAT, 512], BF16, isOutput=False)
    w_qu_d = nc.declare_dram_parameter("w_qu", [QR, 512], BF16, isOutput=False)
    w_o_d = nc.declare_dram_parameter("w_o", [512, DIM], BF16, isOutput=False)
    b_kvc_d = nc.declare_dram_parameter("b_kvc", [LAT, 1], F32, isOutput=False)
    b_qc_d = nc.declare_dram_parameter("b_qc", [128, 2], F32, isOutput=False)
    b_qu_d = nc.declare_dram_parameter("b_qu", [128, 4], F32, isOutput=False)
    b_kvu_k_d = nc.declare_dram_parameter("b_kvu_k", [128, 4], F32, isOutput=False)
    b_kvu_v_d = nc.declare_dram_parameter("b_kvu_v", [128, 512], F32, isOutput=False)
    b_o_d = nc.declare_dram_parameter("b_o", [128, DIM], F32, isOutput=False)
    out_d = nc.declare_dram_parameter("out", [S, DIM], F32, isOutput=True)

    with tile.TileContext(nc) as tc:
        with (
            tc.tile_pool(name="const", bufs=1) as const,
            tc.tile_pool(name="wts", bufs=1) as wts,
            tc.tile_pool(name="big", bufs=1) as big,
            tc.tile_pool(name="xin", bufs=4) as xin,
            tc.tile_pool(name="kvq", bufs=2) as kvq,
            tc.tile_pool(name="exd", bufs=6) as exd,
            tc.tile_pool(name="nrm", bufs=2) as nrm,
            tc.tile_pool(name="outs", bufs=3) as outs,
            tc.tile_pool(name="mps", bufs=2, space="PSUM") as mps,
        ):
            # ---- weight tiles (DMA'd inside W1, after the x DMAs) ----------
            w_kvc_sb = wts.tile([128, DIM], BF16, name="w_kvc_sb")
            w_qc_sb = wts.tile([128, ND * QR], BF16, name="w_qc_sb")
            w_kvu_k_sb = wts.tile([128, 512], BF16, name="w_kvu_k_sb")
            w_kvu_v_sb = wts.tile([128, 512], BF16, name="w_kvu_v_sb")
            w_qu_sb = wts.tile([128, 1024], BF16, name="w_qu_sb")
            w_o_sb = wts.tile([128, 4 * DIM], BF16, name="w_o_sb")
            b_kvc_sb = wts.tile([128, 1], F32, name="b_kvc_sb")
            b_qc_sb = wts.tile([128, 2], F32, name="b_qc_sb")
            b_qu_sb = wts.tile([128, 4], F32, name="b_qu_sb")
            b_kvu_k_sb = wts.tile([128, 4], F32, name="b_kvu_k_sb")
            b_kvu_v_sb = wts.tile([128, 512], F32, name="b_kvu_v_sb")
            b_o_sb = wts.tile([128, DIM], F32, name="b_o_sb")

            def emit_wdma_early():
                # merged DMAs (one instruction each); only what the latent
                # projections need right away.
                nc.sync.dma_start(
                    out=w_kvc_sb[:].rearrange("p (d c) -> p d c", c=128),
                    in_=w_kvc_d[:, :].rearrange("(d p) c -> p d c", p=128))
                nc.sync.dma_start(out=b_kvc_sb[:], in_=b_kvc_d[:, :])
                nc.sync.dma_start(
                    out=w_qc_sb[:].rearrange("p (d c) -> p d c", c=QR),
                    in_=w_qc_d[:, :].rearrange("(d p) c -> p d c", p=128))
                nc.sync.dma_start(out=b_qc_sb[:], in_=b_qc_d[:, :])

            def emit_wdma_mid():
                nc.sync.dma_start(out=w_kvu_k_sb[:], in_=w_kvu_k_d[:, :])
                nc.sync.dma_start(
                    out=w_qu_sb[:].rearrange("p (d c) -> p d c", c=512),
                    in_=w_qu_d[:, :].rearrange("(d p) c -> p d c", p=128))
                nc.sync.dma_start(out=b_qu_sb[:], in_=b_qu_d[:, :])
                nc.sync.dma_start(out=b_kvu_k_sb[:], in_=b_kvu_k_d[:, :])
                nc.sync.dma_start(out=w_kvu_v_sb[:], in_=w_kvu_v_d[:, :])
                nc.sync.dma_start(out=b_kvu_v_sb[:], in_=b_kvu_v_d[:, :])

            def emit_wdma_late():
                nc.sync.dma_start(
                    out=w_o_sb[:].rearrange("p (d c) -> p d c", c=DIM),
                    in_=w_o_d[:, :].rearrange("(d p) c -> p d c", p=128))
                nc.sync.dma_start(out=b_o_sb[:], in_=b_o_d[:, :])

            # ---- persistent products ---------------------------------------
            KT = big.tile([128, 4 * S], BF16, name="KT")
            QT = big.tile([128, 4 * S], BF16, name="QT")
            V = big.tile([128, NT * 520], BF16, name="V")
            ctxT = big.tile([128, 4 * S], BF16, name="ctxT")
            v_view = V[:].rearrange("p (k h c) -> p k h c", h=NHL, c=65)
            # ones columns of V (col 64 of each 65-wide head block)
            ones_f = const.tile([128, NT * NHL], F32, name="ones_f")
            nc.gpsimd.memset(ones_f[:], 1.0)
            nc.vector.tensor_copy(
                v_view[:, :, :, 64:65],
                ones_f[:].rearrange("p (k h o) -> p k h o", h=NHL, o=1))
            # warm the Exp activation table while the DMAs run so the first
            # real exp doesn't pay the table load
            warm = const.tile([1, 1], F32, name="warm")
            nc.scalar.activation(warm[:], ones_f[0:1, 0:1], AF.Exp)

            # ---- projection emitters ---------------------------------------
            xts = [None] * NP
            kvs = [None] * NP
            q0s = [None] * NP
            q1s = [None] * NP

            def emit_xdma(p, dlo=0, dhi=ND):
                if dlo == 0:
                    xts[p] = xin.tile([128, ND * 512], BF16, tag="xt",
                                      name=f"xt{p}")
                xt = xts[p]
                nc.sync.dma_start(
                    out=xt[:, 512 * dlo:512 * dhi].rearrange(
                        "p (d s) -> p d s", s=512),
                    in_=xT_d[128 * dlo:128 * dhi, 512 * p:512 * p + 512]
                    .rearrange("(d p) s -> p d s", p=128))

            def emit_lat_kv(p, pool):
                ps = pool.tile([128, 512], F32, tag="mm")
                for dc in range(ND):
                    nc.tensor.matmul(
                        ps[:], w_kvc_sb[:, 128 * dc:128 * dc + 128],
                        xts[p][:, 512 * dc:512 * dc + 512],
                        start=dc == 0, stop=dc == ND - 1)
                t = kvq.tile([128, 512], BF16, tag="kvs")
                kvs[p] = t
                nc.vector.tensor_scalar_add(t[:], ps[:], b_kvc_sb[:, 0:1])

            def emit_lat_q(p, half, pool):
                ps = pool.tile([128, 512], F32, tag="mm")
                for dc in range(ND):
                    nc.tensor.matmul(
                        ps[:], w_qc_sb[:, QR * dc + 128 * half:QR * dc + 128 * half + 128],
                        xts[p][:, 512 * dc:512 * dc + 512],
                        start=dc == 0, stop=dc == ND - 1)
                t = kvq.tile([128, 512], BF16, tag=f"q{half}s")
                (q0s if half == 0 else q1s)[p] = t
                nc.vector.tensor_scalar_add(t[:], ps[:], b_qc_sb[:, half:half + 1])

            def emit_qt(p, c, pool):
                ps = pool.tile([128, 512], F32, tag="mm")
                nc.tensor.matmul(ps[:], w_qu_sb[:, 128 * c:128 * c + 128],
                                 q0s[p][:], start=True, stop=False)
                nc.tensor.matmul(ps[:], w_qu_sb[:, 512 + 128 * c:512 + 128 * c + 128],
                                 q1s[p][:], start=False, stop=True)
                nc.vector.tensor_scalar_add(
                    QT[:, c * S + 512 * p:c * S + 512 * p + 512], ps[:],
                    b_qu_sb[:, c:c + 1])

            def emit_kt(p, c, pool):
                ps = pool.tile([128, 512], F32, tag="mm")
                nc.tensor.matmul(ps[:], w_kvu_k_sb[:, 128 * c:128 * c + 128],
                                 kvs[p][:], start=True, stop=True)
                nc.vector.tensor_scalar_add(
                    KT[:, c * S + 512 * p:c * S + 512 * p + 512], ps[:],
                    b_kvu_k_sb[:, c:c + 1])

            def emit_v(p, q, pool):
                k = 4 * p + q
                ps = pool.tile([128, 512], F32, tag="mm")
                nc.tensor.matmul(ps[:], kvs[p][:, 128 * q:128 * q + 128],
                                 w_kvu_v_sb[:], start=True, stop=True)
                nc.vector.tensor_add(
                    v_view[:, k, :, 0:64],
                    ps[:].rearrange("p (h c) -> p h c", c=64),
                    b_kvu_v_sb[:].rearrange("p (h c) -> p h c", c=64))

            obs = [None] * 4

            def emit_outproj_half(si, o2, pool):
                if o2 == 0:
                    obs[si % 4] = outs.tile([128, DIM], F32, tag="ob",
                                            name=f"ob{si}")
                ps = pool.tile([128, 512], F32, tag="mm")
                for cc in range(4):
                    nc.tensor.matmul(
                        ps[:],
                        ctxT[:, cc * S + 128 * si:cc * S + 128 * si + 128],
                        w_o_sb[:, DIM * cc + o2:DIM * cc + o2 + 512],
                        start=cc == 0, stop=cc == 3)
                ob = obs[si % 4]
                nc.vector.tensor_add(ob[:, o2:o2 + 512], ps[:],
                                     b_o_sb[:, o2:o2 + 512])
                nc.sync.dma_start(
                    out=out_d[128 * si:128 * si + 128, o2:o2 + 512],
                    in_=ob[:, o2:o2 + 512])

            # ---- attention ------------------------------------------------
            def attn_head(j, h, scp, ctp, fill, need=None):
                """Attention for s-half j, local head h.  PV is emitted one
                k-iteration late so the PE never puts PV(k)/QK(k+1) between
                exp(k) and exp(k+1) on the critical path — the scalar engine
                (exp) is the attention-phase bottleneck and must never wait.
                fill() interleaves projection/out_proj PE work; need(k) drains
                fills that iteration k depends on."""
                s0 = SH * j
                c = h // 2
                po = 64 * (h % 2)
                kmax = (SH // 128) * (j + 1)
                nbank = SH // 512
                last_k = {bi: min(kmax - 1, (s0 + 512 * (bi + 1)) // 128 - 1)
                          for bi in range(nbank)}
                ctx = ctp.tile([65, SH], F32, tag="ctx")

                def emit_pv(k, rel, ex):
                    for bi in range(nbank):
                        a = max(rel, 512 * bi)
                        b2 = min(SH, 512 * bi + 512)
                        if a >= b2:
                            continue
                        nc.tensor.matmul(
                            ctx[:, a:b2],
                            V[:, 520 * k + 65 * h:520 * k + 65 * h + 65],
                            ex[:, a - rel:b2 - rel],
                            start=(k == 0), stop=(k == last_k[bi]))

                pend = []
                for k in range(kmax):
                    if need is not None:
                        need(k)
                    t0 = 128 * k
                    ss = max(s0, t0)
                    fd = s0 + SH - ss
                    rel = ss - s0
                    sc = scp.tile([128, SH], F32, tag="sc")
                    for o2, w2 in _pieces(fd):
                        nc.tensor.matmul(
                            sc[:, o2:o2 + w2],
                            KT[po:po + 64, c * S + t0:c * S + t0 + 128],
                            QT[po:po + 64, c * S + ss + o2:c * S + ss + o2 + w2],
                            start=True, stop=True)
                    ex = exd.tile([128, SH], BF16, tag="ex")
                    nc.scalar.activation(ex[:, :fd], sc[:, :fd], AF.Exp,
                                         scale=0.125)
                    if t0 >= s0:
                        nc.gpsimd.affine_select(
                            out=ex[:, 0:128], in_=ex[:, 0:128],
                            pattern=[[1, 128]],
                            compare_op=mybir.AluOpType.is_ge,
                            fill=0.0, base=0, channel_multiplier=-1)
                    if len(pend) == 2:
                        emit_pv(*pend.pop(0))
                    pend.append((k, rel, ex))
                    fill()
                for a in pend:
                    emit_pv(*a)
                # normalize: ctx[0:64] * (1/ctx[64]) -> ctxT, in two
                # column-halves so the ctx PSUM slot releases sooner
                for o2 in (0, SH // 2):
                    hs = SH // 2
                    rec = nrm.tile([1, SH // 2], F32, tag="rec")
                    nc.vector.reciprocal(rec[:], ctx[64:65, o2:o2 + hs])
                    rbc = nrm.tile([64, SH // 2], F32, tag="rbc")
                    nc.gpsimd.partition_broadcast(rbc[:], rec[0:1, :])
                    nc.vector.tensor_mul(
                        ctxT[po:po + 64, c * S + s0 + o2:c * S + s0 + o2 + hs],
                        ctx[0:64, o2:o2 + hs], rbc[:])

            # ================= W1: minimal projection prefix =================
            # Just enough that head 0's j=0 attention can start: q-latents for
            # pieces 0,1, the c=0 K/Q up-projections, V chunk 0.  Everything
            # else becomes interleaved fill work.
            with tc.tile_pool(name="w1p", bufs=4, space="PSUM") as w1p:
                emit_xdma(0, 0, 4)
                emit_wdma_early()
                emit_xdma(0, 4, ND)
                emit_xdma(1)
                emit_wdma_mid()
                emit_xdma(2)
                emit_xdma(3)
                emit_wdma_late()
                emit_lat_kv(0, w1p)
                emit_lat_q(0, 0, w1p)
                emit_lat_q(0, 1, w1p)
                emit_lat_q(1, 0, w1p)
                emit_lat_q(1, 1, w1p)
                emit_kt(0, 0, w1p)
                emit_qt(0, 0, w1p)
                emit_qt(1, 0, w1p)
                emit_v(0, 0, w1p)

            # Unified interleaved fill stream, in dependency order; labels
            # mark the last step each attention point requires.  The j=0
            # window drains only up to CAP (its own needs); the p2/p3
            # projections and j=0 out_proj drain lazily through the j=1
            # window, where the scalar engine is the bottleneck and the PE
            # has idle slack.
            fills = []

            def F(label, fn):
                fills.append((label, fn))

            F("h0i1", lambda: emit_lat_kv(1, mps))
            F("h0i2", lambda: emit_v(0, 1, mps))
            F("h0i3", lambda: emit_v(0, 2, mps))
            F("h0i4", lambda: emit_kt(1, 0, mps))
            F("h0i4", lambda: emit_v(0, 3, mps))
            F("h0i5", lambda: emit_v(1, 0, mps))
            F("h0i6", lambda: emit_v(1, 1, mps))
            F("h0i7", lambda: emit_v(1, 2, mps))
            F("h0i7", lambda: emit_v(1, 3, mps))
            for c in (1, 2, 3):
                for p in (0, 1):
                    F(f"kq{c}", lambda p=p, c=c: emit_qt(p, c, mps))
                    F(f"kq{c}", lambda p=p, c=c: emit_kt(p, c, mps))
            for p in (2, 3):
                F("j1lat", lambda p=p: emit_lat_kv(p, mps))
                F("j1lat", lambda p=p: emit_lat_q(p, 0, mps))
                F("j1lat", lambda p=p: emit_lat_q(p, 1, mps))
            F("j1q0", lambda: emit_qt(2, 0, mps))
            F("j1q0", lambda: emit_qt(3, 0, mps))
            F("j1k8c0", lambda: emit_kt(2, 0, mps))
            for q in range(4):
                F("j1v2", lambda q=q: emit_v(2, q, mps))
            F("j1k12c0", lambda: emit_kt(3, 0, mps))
            for q in range(4):
                F("j1v3", lambda q=q: emit_v(3, q, mps))
            for c in (1, 2, 3):
                F(f"j1q{c}", lambda c=c: emit_qt(2, c, mps))
                F(f"j1q{c}", lambda c=c: emit_qt(3, c, mps))
                F(f"j1k8c{c}", lambda c=c: emit_kt(2, c, mps))
                F(f"j1k12c{c}", lambda c=c: emit_kt(3, c, mps))
            for si in range(8):
                F("op0", lambda si=si: emit_outproj_half(si, 0, mps))
                F("op0", lambda si=si: emit_outproj_half(si, 512, mps))

            fill_pos = [0]
            CAP = max(i for i, (lb, _) in enumerate(fills) if lb == "j1q0") + 1

            def drain(n, cap=None):
                lim = len(fills) if cap is None else cap
                while n > 0 and fill_pos[0] < lim:
                    fills[fill_pos[0]][1]()
                    fill_pos[0] += 1
                    n -= 1

            def drain_until(label):
                idx = max((i for i, (lb, _) in enumerate(fills) if lb == label),
                          default=-1)
                while fill_pos[0] <= idx:
                    fills[fill_pos[0]][1]()
                    fill_pos[0] += 1

            with (
                tc.tile_pool(name="scp", bufs=2, space="PSUM") as scp,
                tc.tile_pool(name="ctp", bufs=1, space="PSUM") as ctp,
            ):
                # ================= W2: attention j=0 ========================
                def need_h0(k):
                    drain_until(f"h0i{k}")

                for h in range(NHL):
                    if h >= 2:
                        drain_until(f"kq{h // 2}")
                    if h == 1:
                        drain_until("h0i7")
                    attn_head(0, h, scp, ctp, lambda: drain(1, cap=CAP),
                              need=need_h0 if h == 0 else None)

                # ================= W3: attention j=1 ========================
                def need_j1(h):
                    c = h // 2

                    def need(k):
                        if k == 0:
                            drain_until(f"j1q{c}")
                        elif k == 8:
                            drain_until("j1v2" if c == 0 else f"j1k8c{c}")
                        elif k == 12:
                            drain_until("j1v3" if c == 0 else f"j1k12c{c}")
                    return need

                for h in range(NHL):
                    attn_head(1, h, scp, ctp, lambda: drain(1),
                              need=need_j1(h))
                drain(len(fills))

            # ================= W4: out_proj j1 tail =========================
            for si in range(8, 16):
                emit_outproj_half(si, 0, mps)
                emit_outproj_half(si, 512, mps)

    nc.finalize()
    return nc


def shard_inputs(inputs, S=2048):
    """Build the 8 per-core input maps from full inputs (host-side prep:
    transpose x, cast matmul operands to bf16, pre-broadcast row biases)."""
    import ml_dtypes
    bf = lambda a: np.ascontiguousarray(np.asarray(a)).astype(ml_dtypes.bfloat16)
    f = lambda a: np.ascontiguousarray(np.asarray(a, dtype=np.float32))
    x = np.asarray(inputs["x"], dtype=np.float32)
    w_kvc, b_kvc = inputs["w_kvc"], f(inputs["b_kvc"])
    w_kvu, b_kvu = np.asarray(inputs["w_kvu"]), f(inputs["b_kvu"])
    w_qc, b_qc = inputs["w_qc"], f(inputs["b_qc"])
    w_qu, b_qu = np.asarray(inputs["w_qu"]), f(inputs["b_qu"])
    w_o, b_o = np.asarray(inputs["w_o"]), f(inputs["b_o"])
    xT = [bf(x[b].T) for b in range(B)]
    w_kvc_b = bf(w_kvc)
    w_qc_b = bf(w_qc)
    in_maps = []
    for core in range(NCORES):
        b = core // 2
        g = core % 2
        cs = slice(512 * g, 512 * g + 512)
        in_maps.append({
            "xT": xT[b],
            "w_kvc": w_kvc_b,
            "w_qc": w_qc_b,
            "w_kvu_k": bf(w_kvu[:, cs]),
            "w_kvu_v": bf(w_kvu[:, 1024 + 512 * g:1024 + 512 * g + 512]),
            "w_qu": bf(w_qu[:, cs]),
            "w_o": bf(w_o[cs, :]),
            "b_kvc": b_kvc.reshape(LAT, 1),
            "b_qc": np.ascontiguousarray(b_qc.reshape(2, 128).T),
            "b_qu": np.ascontiguousarray(b_qu[cs].reshape(4, 128).T),
            "b_kvu_k": np.ascontiguousarray(b_kvu[cs].reshape(4, 128).T),
            "b_kvu_v": np.ascontiguousarray(np.broadcast_to(
                b_kvu[1024 + 512 * g:1024 + 512 * g + 512], (128, 512))),
            "b_o": np.ascontiguousarray(np.broadcast_to(b_o * 0.5, (128, DIM))),
        })
    return in_maps


def kernel(**inputs) -> np.ndarray:
    from concourse.bass_utils import run_bass_kernel_spmd

    x = np.asarray(inputs["x"])
    S = x.shape[1]
    nc = build_mla(S=S)
    in_maps = shard_inputs(inputs, S=S)
    res = run_bass_kernel_spmd(nc, in_maps, list(range(NCORES))).results
    out = np.empty((B, S, DIM), dtype=np.float32)
    for b in range(B):
        out[b] = res[2 * b]["out"] + res[2 * b + 1]["out"]
    return out
